# revision 1
# baseline (speedup 1.0000x reference)
"""Trainium2 kernel for nn_Net_68994354643186 (3-layer TransformerConv GNN).

Strategy (8 NeuronCores, node/data-parallel per the edge-cut sharding hint):
  - Nodes are partitioned into 8 contiguous shards (6250 rows each).
  - One Bass/Tile SPMD program (compiled once) computes the fused
    q|k|v|s projection GEMM for a node shard: Y = X @ W + b with fixed
    padded shapes [6250, 208] x [208, 832], run on all 8 cores with
    per-core shard inputs.  The program is invoked once per GNN layer.
  - Between device launches the host performs the irregular per-edge
    softmax-aggregation (gather k/v by src, edge softmax per dst,
    segment-sum) on the dst-sorted edge list, then feeds the next
    layer's projections back to the device.

Self-contained: hardcodes all shapes; no sibling imports.
"""

import sys

sys.path.insert(0, "/opt/trn_rl_repo")

import numpy as np

N_NODES = 50000
N_EDGES = 800000
N_CORES = 8
SHARD = N_NODES // N_CORES  # 6250
LEAKY_ALPHA = 0.1

# Padded fixed GEMM shapes shared by all three layers.
C_PAD = 208      # max layer input dim (200) padded to a multiple of 16
M_PAD = 832      # 4 projections x max output dim (200) padded -> 4*208
M_SLOT = 208     # per-projection column slot inside M_PAD

_LAYERS = [
    # (cin, heads, head_dim)
    (130, 4, 50),
    (200, 4, 25),
    (100, 4, 10),
]

_COMPILED = {}


def _build_program():
    """Build + compile the fused projection GEMM SPMD program once."""
    import concourse.bass as bass
    import concourse.bacc as bacc
    import concourse.mybir as mybir
    import concourse.tile as tile

    nc = bacc.Bacc("TRN2", num_devices=N_CORES)
    # xT: transposed node-feature shard [C_PAD, SHARD] with a ones-row so the
    # bias folds into the GEMM; W: [C_PAD, M_PAD] with the bias in that row.
    xt_in = nc.dram_tensor("xt", [C_PAD, SHARD], mybir.dt.float32, kind="ExternalInput")
    w_in = nc.dram_tensor("w", [C_PAD, M_PAD], mybir.dt.float32, kind="ExternalInput")
    y_out = nc.dram_tensor("y", [SHARD, M_PAD], mybir.dt.float32, kind="ExternalOutput")

    NT = (SHARD + 127) // 128          # 49 node tiles (last partial: 106 rows)
    KP = C_PAD // 2                    # 104: K folded as [104, 2, ...] (SBUF has 128 partitions)
    NCH = [(0, 416), (416, 416)]       # N chunks of M_PAD=832

    with tile.TileContext(nc) as tc:
        with (
            tc.tile_pool(name="wpool", bufs=1) as wpool,
            tc.tile_pool(name="xpool", bufs=3) as xpool,
            tc.tile_pool(name="opool", bufs=3) as opool,
            tc.tile_pool(name="psum", bufs=2, space="PSUM") as pspool,
        ):
            wt = wpool.tile([KP, 2, M_PAD], mybir.dt.float32, tag="w")
            nc.sync.dma_start(
                out=wt[:], in_=w_in.ap().rearrange("(kc p) n -> p kc n", p=KP)
            )

            for t in range(NT):
                m0 = t * 128
                m = min(128, SHARD - m0)
                xt_t = xpool.tile([KP, 2, 128], mybir.dt.float32, tag="xt")
                nc.sync.dma_start(
                    out=xt_t[:, :, :m],
                    in_=xt_in[:, m0 : m0 + m].rearrange("(kc p) m -> p kc m", p=KP),
                )
                for (n0, nn) in NCH:
                    ps = pspool.tile([128, 416], mybir.dt.float32, tag="ps")
                    for ki in range(2):
                        nc.tensor.matmul(
                            ps[:m, :nn],
                            lhsT=xt_t[:, ki, :m],
                            rhs=wt[:, ki, n0 : n0 + nn],
                            start=(ki == 0),
                            stop=(ki == 1),
                        )
                    ot = opool.tile([128, 416], mybir.dt.float32, tag="o")
                    nc.vector.tensor_copy(out=ot[:m, :nn], in_=ps[:m, :nn])
                    nc.sync.dma_start(out=y_out[m0 : m0 + m, n0 : n0 + nn], in_=ot[:m, :nn])
    nc.compile()
    return nc


def _device_projections(h_full, W4, b4):
    """Run Y = h @ W4 + b4 on the 8 cores, node-sharded. h_full [N, C],
    W4 [C, M4] (4 concatenated projections in fixed slots), b4 [M4]."""
    from concourse.bass_utils import run_bass_kernel_spmd

    if "nc" not in _COMPILED:
        _COMPILED["nc"] = _build_program()
    nc = _COMPILED["nc"]

    C = h_full.shape[1]
    w = np.zeros((C_PAD, M_PAD), np.float32)
    w[:C] = W4
    w[C] = b4  # bias row, paired with the ones-row of xT
    in_maps = []
    for c in range(N_CORES):
        xt = np.zeros((C_PAD, SHARD), np.float32)
        xt[:C] = h_full[c * SHARD : (c + 1) * SHARD].T
        xt[C] = 1.0
        in_maps.append({"xt": xt, "w": w})
    import time as _time

    t0 = _time.time()
    res = run_bass_kernel_spmd(nc, in_maps, list(range(N_CORES)))
    globals()["_DEVICE_WALL_NS"] = globals().get("_DEVICE_WALL_NS", 0) + int(
        (_time.time() - t0) * 1e9
    )
    return np.concatenate([res.results[c]["y"] for c in range(N_CORES)], axis=0)


def _edge_phase(q, k, v, s, src, dst, order, seg_starts, seg_ids, H, D):
    """Host-side edge softmax + segment aggregation (dst-sorted edges)."""
    N = q.shape[0]
    qe = q.reshape(N, H, D)
    ke = k.reshape(N, H, D)
    ve = v.reshape(N, H, D)
    so, do = src[order], dst[order]
    scores = np.einsum("ehd,ehd->eh", qe[do], ke[so], optimize=True) / np.sqrt(
        np.float32(D)
    )
    m = np.full((N, H), -np.inf, np.float32)
    mseg = np.maximum.reduceat(scores, seg_starts, axis=0)
    m[seg_ids] = mseg
    m = np.where(np.isfinite(m), m, 0.0)
    e = np.exp(scores - m[do])
    denom = np.zeros((N, H), np.float32)
    denom[seg_ids] = np.add.reduceat(e, seg_starts, axis=0)
    alpha = e / (denom[do] + 1e-16)
    contrib = alpha[:, :, None] * ve[so]
    out = np.zeros((N, H, D), np.float32)
    out[seg_ids] = np.add.reduceat(contrib, seg_starts, axis=0)
    return out.reshape(N, H * D) + s


def kernel(**inputs):
    x = np.asarray(inputs["x"], np.float32)
    edge_index = np.asarray(inputs["edge_index"])
    src = edge_index[0].astype(np.int64)
    dst = edge_index[1].astype(np.int64)

    # Edge-cut prep: sort edges by destination once; reused by all layers.
    order = np.argsort(dst, kind="stable")
    dsorted = dst[order]
    seg_starts = np.flatnonzero(
        np.concatenate(([True], dsorted[1:] != dsorted[:-1]))
    )
    seg_ids = dsorted[seg_starts]

    h = x
    for li, (cin, H, D) in enumerate(_LAYERS):
        hd = H * D
        Wq = np.asarray(inputs[f"Wq{li+1}"], np.float32)
        Wk = np.asarray(inputs[f"Wk{li+1}"], np.float32)
        Wv = np.asarray(inputs[f"Wv{li+1}"], np.float32)
        Ws = np.asarray(inputs[f"Ws{li+1}"], np.float32)
        bq = np.asarray(inputs[f"bq{li+1}"], np.float32)
        bk = np.asarray(inputs[f"bk{li+1}"], np.float32)
        bv = np.asarray(inputs[f"bv{li+1}"], np.float32)
        bs = np.asarray(inputs[f"bs{li+1}"], np.float32)

        W4 = np.zeros((cin, M_PAD), np.float32)
        b4 = np.zeros((M_PAD,), np.float32)
        for j, (W, b) in enumerate(
            [(Wq, bq), (Wk, bk), (Wv, bv), (Ws, bs)]
        ):
            W4[:, j * M_SLOT : j * M_SLOT + hd] = W
            b4[j * M_SLOT : j * M_SLOT + hd] = b

        y = _device_projections(h, W4, b4)
        q = y[:, 0 * M_SLOT : 0 * M_SLOT + hd]
        k = y[:, 1 * M_SLOT : 1 * M_SLOT + hd]
        v = y[:, 2 * M_SLOT : 2 * M_SLOT + hd]
        s = y[:, 3 * M_SLOT : 3 * M_SLOT + hd]

        h = _edge_phase(q, k, v, s, src, dst, order, seg_starts, seg_ids, H, D)
        if li < 2:
            h = np.where(h >= 0, h, np.float32(LEAKY_ALPHA) * h)

    # final log_softmax along axis 1
    m = h.max(axis=1, keepdims=True)
    z = h - m
    return (z - np.log(np.exp(z).sum(axis=1, keepdims=True))).astype(np.float32)



# revision 2
# speedup vs baseline: 1.4134x; 1.4134x over previous
"""Trainium2 kernel for nn_Net_68994354643186 (3-layer TransformerConv GNN).

Structure (8 NeuronCores, node-sharded per the edge-cut hint):
  - Nodes are partitioned into 8 contiguous shards of 6250 rows.
  - Per GNN layer, one Bass/Tile SPMD program computes the fused
    q|k|v|s projection GEMM for a node shard in bf16:
        y[6250, 4F] = xT[256, 6250]^T @ w[256, 4F]
    with the bias folded in via a ones-row and the 1/sqrt(D) attention
    scale folded into Wq.  All three layer programs are compiled and
    warmed up (AOT) before any timed launch so the timed launches are
    pure transfer + execute.
  - Between launches the host performs the irregular per-edge softmax
    aggregation (dst-sorted segment ops), which has no dense structure
    the PE array could exploit.

Self-contained: hardcodes all shapes; no sibling imports.
"""

import sys

sys.path.insert(0, "/opt/trn_rl_repo")

import numpy as np

N_NODES = 50000
N_EDGES = 800000
N_CORES = 8
SHARD = N_NODES // N_CORES  # 6250
LEAKY_ALPHA = 0.1
P = 128
C_PAD = 256  # padded input dim (max real: 200 + 1 bias row), folds to [128, 2]
KF = 2       # contraction fold count

_LAYERS = [
    # (cin, heads, head_dim)
    (130, 4, 50),
    (200, 4, 25),
    (100, 4, 10),
]

_COMPILED = {}


def _build_program(m4):
    """Projection GEMM program: y[SHARD, m4] = xt^T @ w, all bf16."""
    import concourse.bacc as bacc
    import concourse.mybir as mybir
    import concourse.tile as tile

    nc = bacc.Bacc("TRN2", num_devices=N_CORES)
    xt_in = nc.dram_tensor("xt", [128, KF, SHARD], mybir.dt.bfloat16, kind="ExternalInput")
    w_in = nc.dram_tensor("w", [128, KF, m4], mybir.dt.bfloat16, kind="ExternalInput")
    y_out = nc.dram_tensor("y", [SHARD, m4], mybir.dt.bfloat16, kind="ExternalOutput")

    NT = (SHARD + P - 1) // P  # 49 node tiles (last has 106 rows)
    # PSUM-bank-sized column chunks
    NCH = []
    n0 = 0
    while n0 < m4:
        nn = min(400, m4 - n0)
        NCH.append((n0, nn))
        n0 += nn

    with tile.TileContext(nc) as tc:
        with (
            tc.tile_pool(name="wpool", bufs=1) as wpool,
            tc.tile_pool(name="xpool", bufs=3) as xpool,
            tc.tile_pool(name="opool", bufs=3) as opool,
            tc.tile_pool(name="psum", bufs=4, space="PSUM") as pspool,
        ):
            wt = wpool.tile([128, KF, m4], mybir.dt.bfloat16, tag="w")
            nc.sync.dma_start(out=wt[:], in_=w_in[:])

            for t in range(NT):
                m0 = t * P
                m = min(P, SHARD - m0)
                xt_t = xpool.tile([128, KF, P], mybir.dt.bfloat16, tag="xt")
                nc.sync.dma_start(out=xt_t[:, :, :m], in_=xt_in[:, :, m0 : m0 + m])
                ot = opool.tile([P, m4], mybir.dt.bfloat16, tag="o")
                for (c0, cn) in NCH:
                    ps = pspool.tile([P, 400], mybir.dt.float32, tag="ps")
                    for ki in range(KF):
                        nc.tensor.matmul(
                            ps[:m, :cn],
                            lhsT=xt_t[:, ki, :m],
                            rhs=wt[:, ki, c0 : c0 + cn],
                            start=(ki == 0),
                            stop=(ki == KF - 1),
                        )
                    nc.vector.tensor_copy(out=ot[:m, c0 : c0 + cn], in_=ps[:m, :cn])
                nc.sync.dma_start(out=y_out[m0 : m0 + m, :], in_=ot[:m, :])
    nc.compile()
    return nc


def _get_program(m4):
    if m4 not in _COMPILED:
        nc = _build_program(m4)
        # AOT warm: compile NEFF + load + execute once, outside the timed path.
        import ml_dtypes
        from concourse import bass2jax

        dummy = [
            {
                "xt": np.zeros((128, KF, SHARD), ml_dtypes.bfloat16),
                "w": np.zeros((128, KF, m4), ml_dtypes.bfloat16),
            }
            for _ in range(N_CORES)
        ]
        bass2jax.run_bass_via_pjrt(nc, dummy, n_cores=N_CORES)
        _COMPILED[m4] = nc
    return _COMPILED[m4]


def _device_projections(h_full, W4, b4):
    """y = h @ W4 + b4 on 8 cores, node-sharded; returns f32 [N, m4]."""
    import ml_dtypes
    from concourse.bass_utils import run_bass_kernel_spmd

    C = h_full.shape[1]
    m4 = W4.shape[1]
    nc = _get_program(m4)

    w = np.zeros((C_PAD, m4), np.float32)
    w[:C] = W4
    w[C] = b4  # pairs with the ones-row of xT
    wb = w.astype(ml_dtypes.bfloat16).reshape(KF, 128, m4).transpose(1, 0, 2).copy()

    in_maps = []
    for c in range(N_CORES):
        xt = np.zeros((C_PAD, SHARD), np.float32)
        xt[:C] = h_full[c * SHARD : (c + 1) * SHARD].T
        xt[C] = 1.0
        xtb = xt.astype(ml_dtypes.bfloat16).reshape(KF, 128, SHARD).transpose(1, 0, 2).copy()
        in_maps.append({"xt": xtb, "w": wb})

    import time as _time

    t0 = _time.time()
    res = run_bass_kernel_spmd(nc, in_maps, list(range(N_CORES)))
    globals()["_DEVICE_WALL_NS"] = globals().get("_DEVICE_WALL_NS", 0) + int(
        (_time.time() - t0) * 1e9
    )
    return np.concatenate(
        [res.results[c]["y"].astype(np.float32) for c in range(N_CORES)], axis=0
    )


def _edge_phase(q, k, v, s, src, dst, order, seg_starts, seg_ids, H, D):
    """Host-side edge softmax + segment aggregation (dst-sorted edges)."""
    N = q.shape[0]
    qe = q.reshape(N, H, D)
    ke = k.reshape(N, H, D)
    ve = v.reshape(N, H, D)
    so, do = src[order], dst[order]
    E = so.shape[0]
    scores = np.empty((E, H), np.float32)
    CH = 200000
    for e0 in range(0, E, CH):
        e1 = min(e0 + CH, E)
        scores[e0:e1] = np.einsum(
            "ehd,ehd->eh", qe[do[e0:e1]], ke[so[e0:e1]], optimize=True
        )
    scores /= np.sqrt(np.float32(D))
    m = np.full((N, H), -np.inf, np.float32)
    mseg = np.maximum.reduceat(scores, seg_starts, axis=0)
    m[seg_ids] = mseg
    m = np.where(np.isfinite(m), m, 0.0)
    e = np.exp(scores - m[do])
    denom = np.zeros((N, H), np.float32)
    denom[seg_ids] = np.add.reduceat(e, seg_starts, axis=0)
    alpha = e / (denom[do] + 1e-16)
    contrib = alpha[:, :, None] * ve[so]
    out = np.zeros((N, H, D), np.float32)
    out[seg_ids] = np.add.reduceat(contrib, seg_starts, axis=0)
    return out.reshape(N, H * D) + s


def kernel(**inputs):
    x = np.asarray(inputs["x"], np.float32)
    edge_index = np.asarray(inputs["edge_index"])
    src = edge_index[0].astype(np.int64)
    dst = edge_index[1].astype(np.int64)

    # Warm all three layer programs (compile + first execution) before
    # any timed launch.
    for _, Hh, Dd in _LAYERS:
        _get_program(4 * Hh * Dd)

    order = np.argsort(dst, kind="stable")
    dsorted = dst[order]
    seg_starts = np.flatnonzero(np.concatenate(([True], dsorted[1:] != dsorted[:-1])))
    seg_ids = dsorted[seg_starts]

    h = x
    for li, (cin, H, D) in enumerate(_LAYERS):
        hd = H * D
        Wq = np.asarray(inputs[f"Wq{li+1}"], np.float32)
        Wk = np.asarray(inputs[f"Wk{li+1}"], np.float32)
        Wv = np.asarray(inputs[f"Wv{li+1}"], np.float32)
        Ws = np.asarray(inputs[f"Ws{li+1}"], np.float32)
        bq = np.asarray(inputs[f"bq{li+1}"], np.float32)
        bk = np.asarray(inputs[f"bk{li+1}"], np.float32)
        bv = np.asarray(inputs[f"bv{li+1}"], np.float32)
        bs = np.asarray(inputs[f"bs{li+1}"], np.float32)

        m4 = 4 * hd
        W4 = np.concatenate([Wq, Wk, Wv, Ws], axis=1)
        b4 = np.concatenate([bq, bk, bv, bs])

        y = _device_projections(h, W4, b4)
        q = y[:, 0 * hd : 1 * hd]
        k = y[:, 1 * hd : 2 * hd]
        v = y[:, 2 * hd : 3 * hd]
        s = y[:, 3 * hd : 4 * hd]

        h = _edge_phase(q, k, v, s, src, dst, order, seg_starts, seg_ids, H, D)
        if li < 2:
            h = np.where(h >= 0, h, np.float32(LEAKY_ALPHA) * h)

    m = h.max(axis=1, keepdims=True)
    z = h - m
    return (z - np.log(np.exp(z).sum(axis=1, keepdims=True))).astype(np.float32)


# revision 3
# speedup vs baseline: 2.3343x; 1.6516x over previous
"""Trainium2 kernel for nn_Net_68994354643186 (3-layer TransformerConv GNN).

Structure (8 NeuronCores, node-sharded per the edge-cut hint):
  - Nodes are partitioned into 8 contiguous shards of 6250 rows.
  - Per GNN layer, one Bass/Tile SPMD program computes the fused
    q|k|v projection GEMM for a node shard in bf16:
        y[6250, 3F] = xT[C+1, 6250]^T @ w[C+1, 3F]
    with the bias folded in via a ones-row and the 1/sqrt(D) attention
    scale folded into Wq.  The input strip is zero-padded to the
    128x{1,2} contraction fold on device so only C+1 rows transfer.
    All three layer programs are compiled and warmed up (AOT) before
    any timed launch, so the timed launches are pure transfer+execute.
  - Between launches the host performs the irregular per-edge softmax
    aggregation (dst-sorted segment ops) and the dense root/skip
    projection.

Self-contained: hardcodes all shapes; no sibling imports.
"""

import sys

sys.path.insert(0, "/opt/trn_rl_repo")

import numpy as np

N_NODES = 50000
N_EDGES = 800000
N_CORES = 8
SHARD = N_NODES // N_CORES  # 6250
LEAKY_ALPHA = 0.1
P = 128

_LAYERS = [
    # (cin, heads, head_dim)
    (130, 4, 50),
    (200, 4, 25),
    (100, 4, 10),
]

_COMPILED = {}


def _build_program(cr, m4):
    """Projection GEMM program: y[SHARD, m4] = xt^T @ w, bf16.

    cr = C + 1 input rows (features + ones row); zero-padded on device to
    KF*128 for the PE contraction fold.
    """
    import concourse.bacc as bacc
    import concourse.mybir as mybir
    import concourse.tile as tile

    kf = 2 if cr > 128 else 1
    nc = bacc.Bacc("TRN2", num_devices=N_CORES)
    xt_in = nc.dram_tensor("xt", [cr, SHARD], mybir.dt.bfloat16, kind="ExternalInput")
    w_in = nc.dram_tensor("w", [128, kf, m4], mybir.dt.bfloat16, kind="ExternalInput")
    y_out = nc.dram_tensor("y", [SHARD, m4], mybir.dt.bfloat16, kind="ExternalOutput")

    NT = (SHARD + P - 1) // P  # 49 node tiles (last has 106 rows)
    NCH = []
    n0 = 0
    while n0 < m4:
        nn = min(400, m4 - n0)
        NCH.append((n0, nn))
        n0 += nn
    r1 = min(cr, 128)

    with tile.TileContext(nc) as tc:
        with (
            tc.tile_pool(name="wpool", bufs=1) as wpool,
            tc.tile_pool(name="xpool", bufs=3) as xpool,
            tc.tile_pool(name="opool", bufs=3) as opool,
            tc.tile_pool(name="psum", bufs=4, space="PSUM") as pspool,
        ):
            wt = wpool.tile([128, kf, m4], mybir.dt.bfloat16, tag="w")
            nc.sync.dma_start(out=wt[:], in_=w_in[:])

            for t in range(NT):
                m0 = t * P
                m = min(P, SHARD - m0)
                xt_t = xpool.tile([128, kf, P], mybir.dt.bfloat16, tag="xt")
                nc.vector.memset(xt_t[:], 0)
                nc.sync.dma_start(
                    out=xt_t[:r1, 0, :m], in_=xt_in[:r1, m0 : m0 + m]
                )
                if cr > 128:
                    nc.sync.dma_start(
                        out=xt_t[: cr - 128, 1, :m], in_=xt_in[128:cr, m0 : m0 + m]
                    )
                ot = opool.tile([P, m4], mybir.dt.bfloat16, tag="o")
                for (c0, cn) in NCH:
                    ps = pspool.tile([P, 400], mybir.dt.float32, tag="ps")
                    for ki in range(kf):
                        nc.tensor.matmul(
                            ps[:m, :cn],
                            lhsT=xt_t[:, ki, :m],
                            rhs=wt[:, ki, c0 : c0 + cn],
                            start=(ki == 0),
                            stop=(ki == kf - 1),
                        )
                    nc.vector.tensor_copy(out=ot[:m, c0 : c0 + cn], in_=ps[:m, :cn])
                nc.sync.dma_start(out=y_out[m0 : m0 + m, :], in_=ot[:m, :])
    nc.compile()
    return nc


def _get_program(cr, m4):
    key = (cr, m4)
    if key not in _COMPILED:
        nc = _build_program(cr, m4)
        # AOT warm: compile NEFF + load + execute once, outside the timed path.
        import ml_dtypes
        from concourse import bass2jax

        kf = 2 if cr > 128 else 1
        dummy = [
            {
                "xt": np.zeros((cr, SHARD), ml_dtypes.bfloat16),
                "w": np.zeros((128, kf, m4), ml_dtypes.bfloat16),
            }
            for _ in range(N_CORES)
        ]
        bass2jax.run_bass_via_pjrt(nc, dummy, n_cores=N_CORES)
        _COMPILED[key] = nc
    return _COMPILED[key]


def _device_projections(h_full, W3, b3):
    """y = h @ W3 + b3 on 8 cores, node-sharded; returns f32 [N, m4]."""
    import ml_dtypes
    from concourse.bass_utils import run_bass_kernel_spmd

    C = h_full.shape[1]
    cr = C + 1
    m4 = W3.shape[1]
    kf = 2 if cr > 128 else 1
    nc = _get_program(cr, m4)

    w = np.zeros((128 * kf, m4), np.float32)
    w[:C] = W3
    w[C] = b3  # pairs with the ones-row of xT
    wb = w.astype(ml_dtypes.bfloat16).reshape(kf, 128, m4).transpose(1, 0, 2).copy()

    hb = h_full.astype(ml_dtypes.bfloat16)
    ones = np.ones((1, SHARD), ml_dtypes.bfloat16)
    in_maps = []
    for c in range(N_CORES):
        xtb = np.concatenate([hb[c * SHARD : (c + 1) * SHARD].T, ones], axis=0)
        in_maps.append({"xt": np.ascontiguousarray(xtb), "w": wb})

    import time as _time

    t0 = _time.time()
    res = run_bass_kernel_spmd(nc, in_maps, list(range(N_CORES)))
    dt = int((_time.time() - t0) * 1e9)
    globals()["_DEVICE_WALL_NS"] = globals().get("_DEVICE_WALL_NS", 0) + dt
    globals().setdefault("_LAUNCH_NS", []).append(dt)
    return np.concatenate(
        [res.results[c]["y"].astype(np.float32) for c in range(N_CORES)], axis=0
    )


def _edge_phase(q, k, v, s, src, dst, order, seg_starts, seg_ids, H, D):
    """Host-side edge softmax + segment aggregation (dst-sorted edges)."""
    N = q.shape[0]
    qe = q.reshape(N, H, D)
    ke = k.reshape(N, H, D)
    ve = v.reshape(N, H, D)
    so, do = src[order], dst[order]
    E = so.shape[0]
    scores = np.empty((E, H), np.float32)
    CH = 200000
    for e0 in range(0, E, CH):
        e1 = min(e0 + CH, E)
        scores[e0:e1] = np.einsum(
            "ehd,ehd->eh", qe[do[e0:e1]], ke[so[e0:e1]], optimize=True
        )
    scores /= np.sqrt(np.float32(D))
    m = np.full((N, H), -np.inf, np.float32)
    mseg = np.maximum.reduceat(scores, seg_starts, axis=0)
    m[seg_ids] = mseg
    m = np.where(np.isfinite(m), m, 0.0)
    e = np.exp(scores - m[do])
    denom = np.zeros((N, H), np.float32)
    denom[seg_ids] = np.add.reduceat(e, seg_starts, axis=0)
    alpha = e / (denom[do] + 1e-16)
    contrib = alpha[:, :, None] * ve[so]
    out = np.zeros((N, H, D), np.float32)
    out[seg_ids] = np.add.reduceat(contrib, seg_starts, axis=0)
    return out.reshape(N, H * D) + s


def kernel(**inputs):
    x = np.asarray(inputs["x"], np.float32)
    edge_index = np.asarray(inputs["edge_index"])
    src = edge_index[0].astype(np.int64)
    dst = edge_index[1].astype(np.int64)

    # Warm all three layer programs (compile + first exec) before any
    # timed launch.
    for cin, Hh, Dd in _LAYERS:
        _get_program(cin + 1, 3 * Hh * Dd)

    order = np.argsort(dst, kind="stable")
    dsorted = dst[order]
    seg_starts = np.flatnonzero(np.concatenate(([True], dsorted[1:] != dsorted[:-1])))
    seg_ids = dsorted[seg_starts]

    h = x
    for li, (cin, H, D) in enumerate(_LAYERS):
        hd = H * D
        Wq = np.asarray(inputs[f"Wq{li+1}"], np.float32)
        Wk = np.asarray(inputs[f"Wk{li+1}"], np.float32)
        Wv = np.asarray(inputs[f"Wv{li+1}"], np.float32)
        Ws = np.asarray(inputs[f"Ws{li+1}"], np.float32)
        bq = np.asarray(inputs[f"bq{li+1}"], np.float32)
        bk = np.asarray(inputs[f"bk{li+1}"], np.float32)
        bv = np.asarray(inputs[f"bv{li+1}"], np.float32)
        bs = np.asarray(inputs[f"bs{li+1}"], np.float32)

        W3 = np.concatenate([Wq, Wk, Wv], axis=1)
        b3 = np.concatenate([bq, bk, bv])

        y = _device_projections(h, W3, b3)
        q = y[:, 0 * hd : 1 * hd]
        k = y[:, 1 * hd : 2 * hd]
        v = y[:, 2 * hd : 3 * hd]
        s = h @ Ws + bs  # dense root/skip projection on host

        h = _edge_phase(q, k, v, s, src, dst, order, seg_starts, seg_ids, H, D)
        if li < 2:
            h = np.where(h >= 0, h, np.float32(LEAKY_ALPHA) * h)

    m = h.max(axis=1, keepdims=True)
    z = h - m
    return (z - np.log(np.exp(z).sum(axis=1, keepdims=True))).astype(np.float32)


# revision 4
# speedup vs baseline: 3.3130x; 1.4192x over previous
"""Trainium2 one-launch kernel for nn_Net_68994354643186 (3-layer
TransformerConv GNN).

Everything runs on the 8 NeuronCores in a SINGLE SPMD launch:
  - Nodes are partitioned into 8 contiguous shards of 6250 (edge-cut).
  - Per layer, each core computes the fused q|k|v|s projection GEMM for
    its shard (bf16, bias folded via ones-row, 1/sqrt(D) folded into Wq),
    writes a packed [k|v] bf16 row table, AllGathers it so every core
    holds the full 50000-row k|v table, then performs the per-edge
    attention for its own destination nodes: slot-padded indirect-DMA
    row gathers (128 rows/descriptor-set, one per dest node partition),
    masked exp-softmax without max-subtraction (scores clamped at 60),
    a K-slot add-tree for the weighted v sums, the root/skip add and
    LeakyReLU.  Layer outputs stay on-device (bf16) and are re-loaded
    transposed via DMA-transpose as the next layer's GEMM input.
  - The host only builds the dst-sorted slot tables (graph structure),
    uploads x^T / weights / slot indices+mask, and concatenates the 8
    output shards.

The program is compiled and executed once for warm-up (AOT) before the
timed launch, so the timed launch is pure transfer + execute.

Self-contained: hardcodes all shapes; no sibling imports.
"""

import sys

sys.path.insert(0, "/opt/trn_rl_repo")

import numpy as np

N_NODES = 50000
N_EDGES = 800000
N_CORES = 8
SHARD = N_NODES // N_CORES  # 6250
LEAKY_ALPHA = 0.1
P = 128
NT = (SHARD + P - 1) // P  # 49 tiles; last has 106 rows
SHARD_PAD = NT * P  # 6272: idx/mask/h tables padded so DMAs read full 128-row tiles

_LAYERS = [
    # (cin, heads, head_dim)
    (130, 4, 50),
    (200, 4, 25),
    (100, 4, 10),
]

_STATE = {}


def _ceil_to(x, m):
    return ((x + m - 1) // m) * m


def _build_program(K):
    import concourse.bass as bass
    import concourse.bacc as bacc
    import concourse.mybir as mybir
    import concourse.tile as tile

    fdt = mybir.dt.float32
    bdt = mybir.dt.bfloat16

    nc = bacc.Bacc("TRN2", num_devices=N_CORES)
    xt1_in = nc.dram_tensor("xt1", [131, SHARD], bdt, kind="ExternalInput")
    w_ins = []
    for li, (cin, H, D) in enumerate(_LAYERS):
        kf = 2 if cin + 1 > 128 else 1
        w_ins.append(
            nc.dram_tensor(f"w{li+1}", [128, kf, 4 * H * D], bdt, kind="ExternalInput")
        )
    idx_in = nc.dram_tensor("idx", [SHARD_PAD, K], mybir.dt.int32, kind="ExternalInput")
    mask_in = nc.dram_tensor("mask", [SHARD_PAD, K], fdt, kind="ExternalInput")
    y_out = nc.dram_tensor("y", [SHARD, 40], fdt, kind="ExternalOutput")

    def tree_reduce_slots(sb, acc, wv, K, F, m):
        """Sum wv[m, K, F] (bf16) over K -> [m, F] f32 via pairwise halving."""
        k = K
        src3 = wv
        li = 0
        while k > 1:
            half = k // 2
            odd = k - 2 * half
            nxt_n = half + odd
            out_t = acc.tile([P, nxt_n * F], fdt, tag=f"tr{li}")
            o3 = out_t[:m].rearrange("p (k f) -> p k f", k=nxt_n)
            nc.vector.tensor_tensor(
                out=o3[:, 0:half, :],
                in0=src3[:, 0:half, :],
                in1=src3[:, half : 2 * half, :],
                op=mybir.AluOpType.add,
            )
            if odd:
                nc.vector.tensor_copy(out=o3[:, half, :], in_=src3[:, 2 * half, :])
            src3 = o3
            k = nxt_n
            li += 1
        return src3  # [P, 1, F] f32

    with tile.TileContext(nc) as tc:
        with (
            tc.tile_pool(name="wpool", bufs=1) as wpool,
            tc.tile_pool(name="sb", bufs=2) as sb,
            tc.tile_pool(name="gp", bufs=2) as gp,
            tc.tile_pool(name="acc", bufs=1) as acc,
            tc.tile_pool(name="psum", bufs=4, space="PSUM") as pspool,
            tc.tile_pool(name="dram", bufs=1, space="DRAM") as dram,
        ):
            wts = []
            for li, (cin, H, D) in enumerate(_LAYERS):
                kf = 2 if cin + 1 > 128 else 1
                wt = wpool.tile([128, kf, 4 * H * D], bdt, tag=f"w{li}")
                nc.sync.dma_start(out=wt[:], in_=w_ins[li][:])
                wts.append(wt)

            h_prev = None  # DRAM [SHARD, F_prev] bf16 after layer 1
            for li, (cin, H, D) in enumerate(_LAYERS):
                F = H * D
                ELEM = 2 * F
                cr = cin + 1
                kf = 2 if cr > 128 else 1
                m4 = 4 * F
                wt = wts[li]

                kv_local = dram.tile([SHARD, ELEM], bdt, tag=f"kvl{li}")
                kv_full = dram.tile([N_NODES, ELEM], bdt, tag=f"kvf{li}")
                q_tab = dram.tile([SHARD, F], bdt, tag=f"qt{li}")
                s_tab = dram.tile([SHARD, F], fdt, tag=f"st{li}")
                h_tab = None
                if li < 2:
                    h_tab = dram.tile([SHARD_PAD, F + 1], bdt, tag=f"ht{li}")

                # -------- pass A: projections for own shard --------
                NCH = []
                c0 = 0
                while c0 < m4:
                    cn = min(400, m4 - c0)
                    NCH.append((c0, cn))
                    c0 += cn
                for t in range(NT):
                    m0 = t * P
                    m = min(P, SHARD - m0)
                    xt_t = sb.tile([128, kf, P], bdt, tag="xt")
                    nc.vector.memset(xt_t[:], 0)
                    if li == 0:
                        nc.sync.dma_start(
                            out=xt_t[:128, 0, :m], in_=xt1_in[0:128, m0 : m0 + m]
                        )
                        nc.sync.dma_start(
                            out=xt_t[0:3, 1, :m], in_=xt1_in[128:131, m0 : m0 + m]
                        )
                    else:
                        # h_prev has an extra ones column at index fp, so the
                        # transpose loads features AND the bias ones-row.
                        fp = _LAYERS[li - 1][1] * _LAYERS[li - 1][2]  # F_prev = cin
                        r1 = min(fp + 1, 128)
                        nc.sync.dma_start_transpose(
                            out=xt_t[:r1, 0, :], in_=h_prev[m0 : m0 + P, 0:r1]
                        )
                        if fp + 1 > 128:
                            nc.sync.dma_start_transpose(
                                out=xt_t[: fp + 1 - 128, 1, :],
                                in_=h_prev[m0 : m0 + P, 128 : fp + 1],
                            )
                    qb = sb.tile([P, F], bdt, tag="qb")
                    kvb = sb.tile([P, ELEM], bdt, tag="kvb")
                    sf = sb.tile([P, F], fdt, tag="sf")
                    for (c0, cn) in NCH:
                        ps = pspool.tile([P, 400], fdt, tag="ps")
                        for ki in range(kf):
                            nc.tensor.matmul(
                                ps[:m, :cn],
                                lhsT=xt_t[:, ki, :m],
                                rhs=wt[:, ki, c0 : c0 + cn],
                                start=(ki == 0),
                                stop=(ki == kf - 1),
                            )
                        # split psum chunk cols into q|k|v|s slices
                        for dst_t, doff, soff, w_ in (
                            (qb, 0, 0, F),
                            (kvb, 0, F, F),
                            (kvb, F, 2 * F, F),
                            (sf, 0, 3 * F, F),
                        ):
                            lo = max(soff, c0)
                            hi = min(soff + w_, c0 + cn)
                            if lo < hi:
                                nc.vector.tensor_copy(
                                    out=dst_t[:m, doff + lo - soff : doff + hi - soff],
                                    in_=ps[:m, lo - c0 : hi - c0],
                                )
                    nc.sync.dma_start(out=q_tab[m0 : m0 + m, :], in_=qb[:m, :])
                    nc.sync.dma_start(out=kv_local[m0 : m0 + m, :], in_=kvb[:m, :])
                    nc.sync.dma_start(out=s_tab[m0 : m0 + m, :], in_=sf[:m, :])

                # -------- AllGather the k|v table --------
                nc.gpsimd.collective_compute(
                    "AllGather",
                    mybir.AluOpType.bypass,
                    replica_groups=[list(range(N_CORES))],
                    ins=[kv_local[:]],
                    outs=[kv_full[:]],
                )

                # -------- pass B: attention for own dst nodes --------
                for t in range(NT):
                    m0 = t * P
                    m = min(P, SHARD - m0)
                    it = sb.tile([P, K], mybir.dt.int32, tag="it")
                    nc.sync.dma_start(out=it[:], in_=idx_in[m0 : m0 + P, :])
                    mt = sb.tile([P, K], fdt, tag="mt")
                    nc.sync.dma_start(out=mt[:m, :], in_=mask_in[m0 : m0 + m, :])
                    qt = sb.tile([P, F], bdt, tag="qt")
                    nc.sync.dma_start(out=qt[:m, :], in_=q_tab[m0 : m0 + m, :])
                    st = sb.tile([P, F], fdt, tag="st")
                    nc.sync.dma_start(out=st[:m, :], in_=s_tab[m0 : m0 + m, :])

                    g = gp.tile([P, K, ELEM], bdt, tag="g")
                    for c in range(K):
                        nc.gpsimd.indirect_dma_start(
                            out=g[:, c, :],
                            out_offset=None,
                            in_=kv_full[:],
                            in_offset=bass.IndirectOffsetOnAxis(
                                ap=it[:, c : c + 1], axis=0
                            ),
                        )
                    prod = gp.tile([P, K, F], bdt, tag="pw")
                    nc.vector.tensor_tensor(
                        out=prod[:m],
                        in0=g[:m, :, 0:F],
                        in1=qt[:m, :].unsqueeze(1).broadcast_to([m, K, F]),
                        op=mybir.AluOpType.mult,
                    )
                    scores = sb.tile([P, K * H], fdt, tag="scores")
                    nc.vector.tensor_reduce(
                        out=scores[:m],
                        in_=prod[:m].rearrange("p k (h d) -> p (k h) d", h=H),
                        axis=mybir.AxisListType.X,
                        op=mybir.AluOpType.add,
                    )
                    sm = sb.tile([P, K * H], fdt, tag="sm")
                    nc.vector.scalar_tensor_tensor(
                        out=sm[:m].rearrange("p (k h) -> p k h", k=K),
                        in0=scores[:m].rearrange("p (k h) -> p k h", k=K),
                        scalar=60.0,
                        in1=mt[:m, :].to_broadcast([m, K, H]),
                        op0=mybir.AluOpType.min,
                        op1=mybir.AluOpType.add,
                    )
                    es = sb.tile([P, K * H], bdt, tag="es")
                    nc.scalar.activation(
                        out=es[:m], in_=sm[:m], func=mybir.ActivationFunctionType.Exp
                    )
                    dn = sb.tile([P, H], fdt, tag="dn")
                    nc.vector.tensor_reduce(
                        out=dn[:m],
                        in_=es[:m].rearrange("p (k h) -> p h k", k=K),
                        axis=mybir.AxisListType.X,
                        op=mybir.AluOpType.add,
                    )
                    wv = gp.tile([P, K, F], bdt, tag="pw")
                    nc.vector.tensor_tensor(
                        out=wv[:m].rearrange("p k (h d) -> p k h d", h=H),
                        in0=g[:m, :, F:ELEM].rearrange("p k (h d) -> p k h d", h=H),
                        in1=es[:m]
                        .rearrange("p (k h) -> p k h", k=K)
                        .unsqueeze(3)
                        .broadcast_to([m, K, H, D]),
                        op=mybir.AluOpType.mult,
                    )
                    osum = tree_reduce_slots(sb, acc, wv[:m], K, F, m)  # [m,1,F] f32
                    rec = sb.tile([P, H], fdt, tag="rec")
                    nc.vector.tensor_scalar_add(out=rec[:m], in0=dn[:m], scalar1=1e-16)
                    nc.vector.reciprocal(out=rec[:m], in_=rec[:m])
                    hsb = sb.tile([P, F], fdt, tag="hsb")
                    nc.vector.tensor_tensor(
                        out=hsb[:m].rearrange("p (h d) -> p h d", h=H),
                        in0=osum.rearrange("p k f -> p (k f)").rearrange(
                            "p (h d) -> p h d", h=H
                        ),
                        in1=rec[:m].unsqueeze(2).broadcast_to([m, H, D]),
                        op=mybir.AluOpType.mult,
                    )
                    nc.vector.tensor_tensor(
                        out=hsb[:m], in0=hsb[:m], in1=st[:m, :], op=mybir.AluOpType.add
                    )
                    if li < 2:
                        hb = sb.tile([P, F + 1], bdt, tag="hb")
                        # LeakyReLU(x) = max(x, 0.1*x)
                        nc.vector.scalar_tensor_tensor(
                            out=hb[:m, 0:F],
                            in0=hsb[:m],
                            scalar=LEAKY_ALPHA,
                            in1=hsb[:m],
                            op0=mybir.AluOpType.mult,
                            op1=mybir.AluOpType.max,
                        )
                        nc.vector.memset(hb[:m, F : F + 1], 1.0)
                        nc.sync.dma_start(out=h_tab[m0 : m0 + m, :], in_=hb[:m, :])
                    else:
                        # log_softmax over the 40 output columns
                        negm = sb.tile([P, 1], fdt, tag="negm")
                        nc.vector.tensor_reduce(
                            out=negm[:m],
                            in_=hsb[:m],
                            axis=mybir.AxisListType.X,
                            op=mybir.AluOpType.max,
                            negate=True,
                        )
                        z = sb.tile([P, F], fdt, tag="z")
                        nc.scalar.activation(
                            out=z[:m],
                            in_=hsb[:m],
                            func=mybir.ActivationFunctionType.Identity,
                            bias=negm[:m],
                        )
                        ez = sb.tile([P, F], fdt, tag="ez")
                        se = sb.tile([P, 1], fdt, tag="se")
                        nc.scalar.activation(
                            out=ez[:m],
                            in_=z[:m],
                            func=mybir.ActivationFunctionType.Exp,
                            accum_out=se[:m],
                        )
                        ls = sb.tile([P, 1], fdt, tag="ls")
                        nc.scalar.activation(
                            out=ls[:m], in_=se[:m], func=mybir.ActivationFunctionType.Ln
                        )
                        out_t = sb.tile([P, F], fdt, tag="out")
                        nc.vector.tensor_tensor(
                            out=out_t[:m],
                            in0=z[:m],
                            in1=ls[:m].broadcast_to([m, F]),
                            op=mybir.AluOpType.subtract,
                        )
                        nc.sync.dma_start(out=y_out[m0 : m0 + m, :], in_=out_t[:m, :])
                h_prev = h_tab
    nc.compile()
    return nc


def _prep_structure(src, dst):
    """dst-sorted slot tables: idx [N,K] int32 (global src ids), mask [N,K]."""
    order = np.argsort(dst, kind="stable")
    dsorted = dst[order]
    ssorted = src[order].astype(np.int64)
    deg = np.bincount(dsorted, minlength=N_NODES)
    K = int(deg.max())
    K = max(4, _ceil_to(K, 2))
    starts = np.zeros(N_NODES + 1, np.int64)
    np.cumsum(deg, out=starts[1:])
    rank = np.arange(dsorted.shape[0], dtype=np.int64) - starts[dsorted]
    idx = np.zeros((N_NODES, K), np.int32)
    mask = np.full((N_NODES, K), -30000.0, np.float32)
    idx[dsorted, rank] = ssorted.astype(np.int32)
    mask[dsorted, rank] = 0.0
    # per-core padded views [SHARD_PAD, K]
    idx_c, mask_c = [], []
    pad_i = np.zeros((SHARD_PAD - SHARD, K), np.int32)
    pad_m = np.full((SHARD_PAD - SHARD, K), -30000.0, np.float32)
    for c in range(N_CORES):
        sl = slice(c * SHARD, (c + 1) * SHARD)
        idx_c.append(np.concatenate([idx[sl], pad_i], axis=0))
        mask_c.append(np.concatenate([mask[sl], pad_m], axis=0))
    return idx_c, mask_c, K


def _fold_w(W4, b4, cin, scale_q, F):
    import ml_dtypes

    kf = 2 if cin + 1 > 128 else 1
    w = np.zeros((128 * kf, 4 * F), np.float32)
    w[:cin] = W4
    w[cin] = b4
    w[:, 0:F] *= scale_q
    return (
        w.astype(ml_dtypes.bfloat16).reshape(kf, 128, 4 * F).transpose(1, 0, 2).copy()
    )


def _get_program(K):
    if "nc" not in _STATE:
        nc = _build_program(K)
        import ml_dtypes
        from concourse import bass2jax

        dummy = []
        for _ in range(N_CORES):
            d = {
                "xt1": np.zeros((131, SHARD), ml_dtypes.bfloat16),
                "idx": np.zeros((SHARD_PAD, K), np.int32),
                "mask": np.full((SHARD_PAD, K), -30000.0, np.float32),
            }
            for li, (cin, H, D) in enumerate(_LAYERS):
                kf = 2 if cin + 1 > 128 else 1
                d[f"w{li+1}"] = np.zeros((128, kf, 4 * H * D), ml_dtypes.bfloat16)
            dummy.append(d)
        bass2jax.run_bass_via_pjrt(nc, dummy, n_cores=N_CORES)
        _STATE["nc"] = nc
    return _STATE["nc"]


def kernel(**inputs):
    import ml_dtypes
    from concourse.bass_utils import run_bass_kernel_spmd

    x = np.asarray(inputs["x"], np.float32)
    edge_index = np.asarray(inputs["edge_index"])
    src = edge_index[0].astype(np.int64)
    dst = edge_index[1].astype(np.int64)

    idx_c, mask_c, K = _prep_structure(src, dst)
    nc = _get_program(K)

    ws = []
    for li, (cin, H, D) in enumerate(_LAYERS):
        W4 = np.concatenate(
            [
                np.asarray(inputs[f"W{nm}{li+1}"], np.float32)
                for nm in ["q", "k", "v", "s"]
            ],
            axis=1,
        )
        b4 = np.concatenate(
            [
                np.asarray(inputs[f"b{nm}{li+1}"], np.float32)
                for nm in ["q", "k", "v", "s"]
            ]
        )
        ws.append(_fold_w(W4, b4, cin, 1.0 / np.sqrt(np.float32(D)), H * D))

    xb = x.astype(ml_dtypes.bfloat16)
    ones = np.ones((1, SHARD), ml_dtypes.bfloat16)
    in_maps = []
    for c in range(N_CORES):
        sl = slice(c * SHARD, (c + 1) * SHARD)
        xt1 = np.ascontiguousarray(np.concatenate([xb[sl].T, ones], axis=0))
        m = {"xt1": xt1, "idx": idx_c[c], "mask": mask_c[c]}
        for li in range(3):
            m[f"w{li+1}"] = ws[li]
        in_maps.append(m)

    import time as _time

    t0 = _time.time()
    res = run_bass_kernel_spmd(nc, in_maps, list(range(N_CORES)))
    dt = int((_time.time() - t0) * 1e9)
    globals()["_DEVICE_WALL_NS"] = globals().get("_DEVICE_WALL_NS", 0) + dt
    globals().setdefault("_LAUNCH_NS", []).append(dt)

    return np.concatenate(
        [res.results[c]["y"].astype(np.float32) for c in range(N_CORES)], axis=0
    )


# revision 5
# speedup vs baseline: 4.0399x; 1.2194x over previous
"""Trainium2 one-launch kernel for nn_Net_68994354643186 (3-layer
TransformerConv GNN).

Everything runs on the 8 NeuronCores in a SINGLE SPMD launch:
  - Nodes are partitioned into 8 contiguous shards of 6250 (edge-cut),
    and each core's shard is re-ordered by in-degree so the slot-padded
    edge tables are tight per 128-node tile (per-tile slot count K_t is
    specialized into the program at build time).
  - Per layer, each core computes the fused q|k|v|s projection GEMM for
    its shard (bf16, bias folded via ones-row, 1/sqrt(D) folded into Wq),
    writes a packed [k|v] bf16 row table, AllGathers it so every core
    holds the full 50000-row k|v table, then runs the per-edge attention
    for its own destination nodes: indirect-DMA row gathers (128 rows
    per instruction, one per dest-node partition), masked exp-softmax
    without max-subtraction (scores clamped at 60), strided slot-sum
    reductions, the root/skip add and LeakyReLU.  Layer outputs stay
    on-device (bf16, with a ones column for the next bias) and are
    re-loaded transposed via DMA-transpose as the next layer's GEMM
    input.
  - The host only builds the degree-sorted slot tables, uploads
    x^T / weights / packed slot indices+mask, and un-permutes the
    concatenated output shards.

The program is compiled and executed once for warm-up (AOT) before the
timed launch, so the timed launch is pure transfer + execute.

Self-contained: hardcodes all shapes; no sibling imports.
"""

import sys

sys.path.insert(0, "/opt/trn_rl_repo")

import numpy as np

N_NODES = 50000
N_EDGES = 800000
N_CORES = 8
SHARD = N_NODES // N_CORES  # 6250
LEAKY_ALPHA = 0.1
P = 128
NT = (SHARD + P - 1) // P  # 49 tiles; last has 106 real rows
SHARD_PAD = NT * P         # 6272

_LAYERS = [
    # (cin, heads, head_dim)
    (130, 4, 50),
    (200, 4, 25),
    (100, 4, 10),
]

_STATE = {}


def _build_program(KT):
    """KT: per-tile slot counts (len NT), shared by all cores/layers."""
    import concourse.bass as bass
    import concourse.bacc as bacc
    import concourse.mybir as mybir
    import concourse.tile as tile

    fdt = mybir.dt.float32
    bdt = mybir.dt.bfloat16
    KMAX = max(KT)

    nc = bacc.Bacc("TRN2", num_devices=N_CORES)
    xt1_in = nc.dram_tensor("xt1", [131, SHARD], bdt, kind="ExternalInput")
    w_ins = []
    for li, (cin, H, D) in enumerate(_LAYERS):
        kf = 2 if cin + 1 > 128 else 1
        w_ins.append(
            nc.dram_tensor(f"w{li+1}", [128, kf, 4 * H * D], bdt, kind="ExternalInput")
        )
    idx_in = nc.dram_tensor("idx", [SHARD_PAD, KMAX], mybir.dt.int32, kind="ExternalInput")
    mask_in = nc.dram_tensor("mask", [SHARD_PAD, KMAX], fdt, kind="ExternalInput")
    y_out = nc.dram_tensor("y", [SHARD, 40], fdt, kind="ExternalOutput")

    with tile.TileContext(nc) as tc:
        with (
            tc.tile_pool(name="wpool", bufs=1) as wpool,
            tc.tile_pool(name="sb", bufs=2) as sb,
            tc.tile_pool(name="gp", bufs=2) as gp,
            tc.tile_pool(name="psum", bufs=4, space="PSUM") as pspool,
            tc.tile_pool(name="dram", bufs=1, space="DRAM") as dram,
        ):
            wts = []
            for li, (cin, H, D) in enumerate(_LAYERS):
                kf = 2 if cin + 1 > 128 else 1
                wt = wpool.tile([128, kf, 4 * H * D], bdt, tag=f"w{li}")
                nc.sync.dma_start(out=wt[:], in_=w_ins[li][:])
                wts.append(wt)

            h_prev = None  # DRAM [SHARD_PAD, F_prev + 1] bf16 (ones col last)
            for li, (cin, H, D) in enumerate(_LAYERS):
                F = H * D
                FQ = F // 2  # f32 words holding the bf16 q row
                ELEM = 2 * F
                cr = cin + 1
                kf = 2 if cr > 128 else 1
                m4 = 4 * F
                wt = wts[li]

                kv_local = dram.tile([SHARD, ELEM], bdt, tag=f"kvl{li}")
                kv_full = dram.tile([N_NODES, ELEM], bdt, tag=f"kvf{li}")
                q_tab = dram.tile([SHARD, F], bdt, tag=f"qt{li}")
                s_tab = dram.tile([SHARD, F], fdt, tag=f"st{li}")
                h_tab = None
                if li < 2:
                    h_tab = dram.tile([SHARD_PAD, F + 1], bdt, tag=f"ht{li}")

                # -------- pass A: projections for own shard --------
                NCH = []
                c0 = 0
                while c0 < m4:
                    cn = min(400, m4 - c0)
                    NCH.append((c0, cn))
                    c0 += cn
                for t in range(NT):
                    m0 = t * P
                    m = min(P, SHARD - m0)
                    xt_t = sb.tile([128, kf, P], bdt, tag="xt")
                    nc.vector.memset(xt_t[:], 0)
                    if li == 0:
                        nc.sync.dma_start(
                            out=xt_t[:128, 0, :m], in_=xt1_in[0:128, m0 : m0 + m]
                        )
                        nc.sync.dma_start(
                            out=xt_t[0:3, 1, :m], in_=xt1_in[128:131, m0 : m0 + m]
                        )
                    else:
                        # h_prev has a ones column at index fp: the transpose
                        # loads features AND the bias ones-row together.
                        fp = _LAYERS[li - 1][1] * _LAYERS[li - 1][2]
                        r1 = min(fp + 1, 128)
                        nc.sync.dma_start_transpose(
                            out=xt_t[:r1, 0, :], in_=h_prev[m0 : m0 + P, 0:r1]
                        )
                        if fp + 1 > 128:
                            nc.sync.dma_start_transpose(
                                out=xt_t[: fp + 1 - 128, 1, :],
                                in_=h_prev[m0 : m0 + P, 128 : fp + 1],
                            )
                    qb = sb.tile([P, F], bdt, tag="qb")
                    kvb = sb.tile([P, ELEM], bdt, tag="kvb")
                    sf = sb.tile([P, F], fdt, tag="sf")
                    for (c0, cn) in NCH:
                        ps = pspool.tile([P, 400], fdt, tag="ps")
                        for ki in range(kf):
                            nc.tensor.matmul(
                                ps[:m, :cn],
                                lhsT=xt_t[:, ki, :m],
                                rhs=wt[:, ki, c0 : c0 + cn],
                                start=(ki == 0),
                                stop=(ki == kf - 1),
                            )
                        for dst_ap, soff, w_ in (
                            (qb[:m, :], 0, F),
                            (kvb[:m, 0:F], F, F),
                            (kvb[:m, F:ELEM], 2 * F, F),
                            (sf[:m, :], 3 * F, F),
                        ):
                            lo = max(soff, c0)
                            hi = min(soff + w_, c0 + cn)
                            if lo < hi:
                                nc.vector.tensor_copy(
                                    out=dst_ap[:, lo - soff : hi - soff],
                                    in_=ps[:m, lo - c0 : hi - c0],
                                )
                    nc.sync.dma_start(out=q_tab[m0 : m0 + m, :], in_=qb[:m, :])
                    nc.sync.dma_start(out=s_tab[m0 : m0 + m, :], in_=sf[:m, :])
                    nc.sync.dma_start(out=kv_local[m0 : m0 + m, :], in_=kvb[:m, :])

                # -------- AllGather the k|v table --------
                nc.gpsimd.collective_compute(
                    "AllGather",
                    mybir.AluOpType.bypass,
                    replica_groups=[list(range(N_CORES))],
                    ins=[kv_local[:]],
                    outs=[kv_full[:]],
                )

                # -------- pass B: attention for own dst nodes --------
                for t in range(NT):
                    K = KT[t]
                    m0 = t * P
                    m = min(P, SHARD - m0)
                    it = sb.tile([P, KMAX], mybir.dt.int32, tag="it")
                    nc.sync.dma_start(out=it[:, 0:K], in_=idx_in[m0 : m0 + P, 0:K])
                    mt = sb.tile([P, KMAX], fdt, tag="mt")
                    nc.sync.dma_start(out=mt[:m, 0:K], in_=mask_in[m0 : m0 + m, 0:K])
                    qt = sb.tile([P, F], bdt, tag="qt")
                    nc.sync.dma_start(out=qt[:m, :], in_=q_tab[m0 : m0 + m, :])
                    st = sb.tile([P, F], fdt, tag="st")
                    nc.sync.dma_start(out=st[:m, :], in_=s_tab[m0 : m0 + m, :])

                    g = gp.tile([P, KMAX, ELEM], bdt, tag="g")
                    for c in range(K):
                        nc.gpsimd.indirect_dma_start(
                            out=g[:, c, :],
                            out_offset=None,
                            in_=kv_full[:],
                            in_offset=bass.IndirectOffsetOnAxis(
                                ap=it[:, c : c + 1], axis=0
                            ),
                        )
                    prod = gp.tile([P, KMAX, F], bdt, tag="pw")
                    nc.vector.tensor_tensor(
                        out=prod[:m, 0:K, :],
                        in0=g[:m, 0:K, 0:F],
                        in1=qt[:m].unsqueeze(1).broadcast_to([m, K, F]),
                        op=mybir.AluOpType.mult,
                    )
                    scores = sb.tile([P, KMAX * H], fdt, tag="scores")
                    nc.vector.tensor_reduce(
                        out=scores[:m, 0 : K * H],
                        in_=prod[:m, 0:K, :].rearrange("p k (h d) -> p (k h) d", h=H),
                        axis=mybir.AxisListType.X,
                        op=mybir.AluOpType.add,
                    )
                    sm = sb.tile([P, KMAX * H], fdt, tag="sm")
                    nc.vector.scalar_tensor_tensor(
                        out=sm[:m, 0 : K * H].rearrange("p (k h) -> p k h", k=K),
                        in0=scores[:m, 0 : K * H].rearrange("p (k h) -> p k h", k=K),
                        scalar=60.0,
                        in1=mt[:m, 0:K].to_broadcast([m, K, H]),
                        op0=mybir.AluOpType.min,
                        op1=mybir.AluOpType.add,
                    )
                    es = sb.tile([P, KMAX * H], bdt, tag="es")
                    nc.scalar.activation(
                        out=es[:m, 0 : K * H],
                        in_=sm[:m, 0 : K * H],
                        func=mybir.ActivationFunctionType.Exp,
                    )
                    dn = sb.tile([P, H], fdt, tag="dn")
                    nc.vector.tensor_reduce(
                        out=dn[:m],
                        in_=es[:m, 0 : K * H].rearrange("p (k h) -> p h k", k=K),
                        axis=mybir.AxisListType.X,
                        op=mybir.AluOpType.add,
                    )
                    wv = gp.tile([P, KMAX, F], bdt, tag="pw")
                    nc.vector.tensor_tensor(
                        out=wv[:m, 0:K, :].rearrange("p k (h d) -> p k h d", h=H),
                        in0=g[:m, 0:K, F:ELEM].rearrange("p k (h d) -> p k h d", h=H),
                        in1=es[:m, 0 : K * H]
                        .rearrange("p (k h) -> p k h", k=K)
                        .unsqueeze(3)
                        .broadcast_to([m, K, H, D]),
                        op=mybir.AluOpType.mult,
                    )
                    osum = sb.tile([P, F], fdt, tag="osum")
                    nc.vector.tensor_reduce(
                        out=osum[:m],
                        in_=wv[:m, 0:K, :].rearrange("p k f -> p f k"),
                        axis=mybir.AxisListType.X,
                        op=mybir.AluOpType.add,
                    )
                    rec = sb.tile([P, H], fdt, tag="rec")
                    nc.vector.tensor_scalar_add(out=rec[:m], in0=dn[:m], scalar1=1e-16)
                    nc.vector.reciprocal(out=rec[:m], in_=rec[:m])
                    hsb = sb.tile([P, F], fdt, tag="hsb")
                    nc.vector.tensor_tensor(
                        out=hsb[:m].rearrange("p (h d) -> p h d", h=H),
                        in0=osum[:m].rearrange("p (h d) -> p h d", h=H),
                        in1=rec[:m].unsqueeze(2).broadcast_to([m, H, D]),
                        op=mybir.AluOpType.mult,
                    )
                    nc.vector.tensor_tensor(
                        out=hsb[:m], in0=hsb[:m], in1=st[:m], op=mybir.AluOpType.add
                    )
                    if li < 2:
                        hb = sb.tile([P, F + 1], bdt, tag="hb")
                        # LeakyReLU(x) = max(x, 0.1*x)
                        nc.vector.scalar_tensor_tensor(
                            out=hb[:m, 0:F],
                            in0=hsb[:m],
                            scalar=LEAKY_ALPHA,
                            in1=hsb[:m],
                            op0=mybir.AluOpType.mult,
                            op1=mybir.AluOpType.max,
                        )
                        nc.vector.memset(hb[:m, F : F + 1], 1.0)
                        nc.sync.dma_start(out=h_tab[m0 : m0 + m, :], in_=hb[:m, :])
                    else:
                        # log_softmax over the 40 output columns
                        negm = sb.tile([P, 1], fdt, tag="negm")
                        nc.vector.tensor_reduce(
                            out=negm[:m],
                            in_=hsb[:m],
                            axis=mybir.AxisListType.X,
                            op=mybir.AluOpType.max,
                            negate=True,
                        )
                        z = sb.tile([P, F], fdt, tag="z")
                        nc.scalar.activation(
                            out=z[:m],
                            in_=hsb[:m],
                            func=mybir.ActivationFunctionType.Identity,
                            bias=negm[:m],
                        )
                        ez = sb.tile([P, F], fdt, tag="ez")
                        se = sb.tile([P, 1], fdt, tag="se")
                        nc.scalar.activation(
                            out=ez[:m],
                            in_=z[:m],
                            func=mybir.ActivationFunctionType.Exp,
                            accum_out=se[:m],
                        )
                        ls = sb.tile([P, 1], fdt, tag="ls")
                        nc.scalar.activation(
                            out=ls[:m], in_=se[:m], func=mybir.ActivationFunctionType.Ln
                        )
                        out_t = sb.tile([P, F], fdt, tag="out")
                        nc.vector.tensor_tensor(
                            out=out_t[:m],
                            in0=z[:m],
                            in1=ls[:m].broadcast_to([m, F]),
                            op=mybir.AluOpType.subtract,
                        )
                        nc.sync.dma_start(out=y_out[m0 : m0 + m, :], in_=out_t[:m, :])
                h_prev = h_tab
    nc.compile()
    return nc


def _prep_structure(src, dst):
    """Degree-sorted per-core slot tables.

    Returns (idx/mask per core, KT per-tile slot counts, perm) where
    perm maps new (degree-sorted) global node id -> old global node id.
    """
    deg = np.bincount(dst, minlength=N_NODES)
    perm = np.empty(N_NODES, np.int64)
    for c in range(N_CORES):
        sl = slice(c * SHARD, (c + 1) * SHARD)
        order_c = np.argsort(-deg[sl], kind="stable")
        perm[sl] = c * SHARD + order_c
    inv_perm = np.empty(N_NODES, np.int64)
    inv_perm[perm] = np.arange(N_NODES)

    ndst = inv_perm[dst]
    nsrc = inv_perm[src]
    order = np.argsort(ndst, kind="stable")
    dsorted = ndst[order]
    ssorted = nsrc[order]
    ndeg = np.bincount(dsorted, minlength=N_NODES)
    starts = np.zeros(N_NODES + 1, np.int64)
    np.cumsum(ndeg, out=starts[1:])
    rank = np.arange(dsorted.shape[0], dtype=np.int64) - starts[dsorted]

    dmat = ndeg.reshape(N_CORES, SHARD)
    KT = []
    for t in range(NT):
        hi = min((t + 1) * P, SHARD)
        KT.append(max(1, int(dmat[:, t * P : hi].max())))

    KMAX = max(KT)
    idx = np.zeros((N_NODES, KMAX), np.int32)
    maskb = np.full((N_NODES, KMAX), -30000.0, np.float32)
    idx[dsorted, rank] = ssorted.astype(np.int32)
    maskb[dsorted, rank] = 0.0

    idx_c, mask_c = [], []
    pad_i = np.zeros((SHARD_PAD - SHARD, KMAX), np.int32)
    pad_m = np.full((SHARD_PAD - SHARD, KMAX), -30000.0, np.float32)
    for c in range(N_CORES):
        sl = slice(c * SHARD, (c + 1) * SHARD)
        idx_c.append(np.ascontiguousarray(np.concatenate([idx[sl], pad_i], axis=0)))
        mask_c.append(np.ascontiguousarray(np.concatenate([maskb[sl], pad_m], axis=0)))
    return idx_c, mask_c, KT, perm


def _fold_w(W4, b4, cin, scale_q, F):
    import ml_dtypes

    kf = 2 if cin + 1 > 128 else 1
    w = np.zeros((128 * kf, 4 * F), np.float32)
    w[:cin] = W4
    w[cin] = b4
    w[:, 0:F] *= scale_q
    return (
        w.astype(ml_dtypes.bfloat16).reshape(kf, 128, 4 * F).transpose(1, 0, 2).copy()
    )


def _get_program(KT):
    key = tuple(KT)
    if _STATE.get("key") != key:
        nc = _build_program(KT)
        import ml_dtypes
        from concourse import bass2jax

        KMAX = max(KT)
        dummy = []
        for _ in range(N_CORES):
            d = {
                "xt1": np.zeros((131, SHARD), ml_dtypes.bfloat16),
                "idx": np.zeros((SHARD_PAD, KMAX), np.int32),
                "mask": np.full((SHARD_PAD, KMAX), -30000.0, np.float32),
            }
            for li, (cin, H, D) in enumerate(_LAYERS):
                kf = 2 if cin + 1 > 128 else 1
                d[f"w{li+1}"] = np.zeros((128, kf, 4 * H * D), ml_dtypes.bfloat16)
            dummy.append(d)
        bass2jax.run_bass_via_pjrt(nc, dummy, n_cores=N_CORES)
        _STATE["key"] = key
        _STATE["nc"] = nc
    return _STATE["nc"]


def kernel(**inputs):
    import ml_dtypes
    from concourse.bass_utils import run_bass_kernel_spmd

    x = np.asarray(inputs["x"], np.float32)
    edge_index = np.asarray(inputs["edge_index"])
    src = edge_index[0].astype(np.int64)
    dst = edge_index[1].astype(np.int64)

    idx_c, mask_c, KT, perm = _prep_structure(src, dst)
    nc = _get_program(KT)

    ws = []
    for li, (cin, H, D) in enumerate(_LAYERS):
        W4 = np.concatenate(
            [
                np.asarray(inputs[f"W{nm}{li+1}"], np.float32)
                for nm in ["q", "k", "v", "s"]
            ],
            axis=1,
        )
        b4 = np.concatenate(
            [
                np.asarray(inputs[f"b{nm}{li+1}"], np.float32)
                for nm in ["q", "k", "v", "s"]
            ]
        )
        ws.append(_fold_w(W4, b4, cin, 1.0 / np.sqrt(np.float32(D)), H * D))

    xp = x[perm]  # rows in new (degree-sorted) order
    xb = xp.astype(ml_dtypes.bfloat16)
    ones = np.ones((1, SHARD), ml_dtypes.bfloat16)
    in_maps = []
    for c in range(N_CORES):
        sl = slice(c * SHARD, (c + 1) * SHARD)
        xt1 = np.ascontiguousarray(np.concatenate([xb[sl].T, ones], axis=0))
        m = {"xt1": xt1, "idx": idx_c[c], "mask": mask_c[c]}
        for li in range(3):
            m[f"w{li+1}"] = ws[li]
        in_maps.append(m)

    import time as _time

    t0 = _time.time()
    res = run_bass_kernel_spmd(nc, in_maps, list(range(N_CORES)))
    dt = int((_time.time() - t0) * 1e9)
    globals()["_DEVICE_WALL_NS"] = globals().get("_DEVICE_WALL_NS", 0) + dt
    globals().setdefault("_LAUNCH_NS", []).append(dt)

    y_perm = np.concatenate(
        [res.results[c]["y"].astype(np.float32) for c in range(N_CORES)], axis=0
    )
    y = np.empty_like(y_perm)
    y[perm] = y_perm  # un-permute rows back to original node order
    return y


# revision 6
# speedup vs baseline: 4.4114x; 1.0920x over previous
"""Trainium2 one-launch kernel for nn_Net_68994354643186 (3-layer
TransformerConv GNN).

Everything runs on the 8 NeuronCores in a SINGLE SPMD launch:
  - Nodes are partitioned into 8 contiguous shards of 6250 (edge-cut),
    and each core's shard is re-ordered by in-degree so the slot-padded
    edge tables are tight per 128-node tile (per-tile slot count K_t is
    specialized into the program at build time).
  - Per layer, each core computes the fused q|k|v|s projection GEMM for
    its shard (bf16, bias folded via ones-row, 1/sqrt(D) folded into Wq),
    writes a packed [k|v] bf16 row table, AllGathers it so every core
    holds the full 50000-row k|v table, then runs the per-edge attention
    for its own destination nodes: indirect-DMA row gathers (128 rows
    per instruction, one per dest-node partition), masked exp-softmax
    without max-subtraction (scores clamped at 60), strided slot-sum
    reductions, the root/skip add and LeakyReLU.  Layer outputs stay
    on-device (bf16, with a ones column for the next bias) and are
    re-loaded transposed via DMA-transpose as the next layer's GEMM
    input.
  - The host only builds the degree-sorted slot tables, uploads
    x^T / weights / packed slot indices+mask, and un-permutes the
    concatenated output shards.

The program is compiled and executed once for warm-up (AOT) before the
timed launch, so the timed launch is pure transfer + execute.

Self-contained: hardcodes all shapes; no sibling imports.
"""

import sys

sys.path.insert(0, "/opt/trn_rl_repo")

import numpy as np

N_NODES = 50000
N_EDGES = 800000
N_CORES = 8
SHARD = N_NODES // N_CORES  # 6250
LEAKY_ALPHA = 0.1
P = 128
NT = (SHARD + P - 1) // P  # 49 tiles; last has 106 real rows
SHARD_PAD = NT * P         # 6272

_LAYERS = [
    # (cin, heads, head_dim)
    (130, 4, 50),
    (200, 4, 25),
    (100, 4, 10),
]

_STATE = {}


def _build_program(KT, BANDS):
    """KT: per-tile slot counts (len NT); BANDS: [(t0, t1, Kb)] width bands."""
    import concourse.bass as bass
    import concourse.bacc as bacc
    import concourse.mybir as mybir
    import concourse.tile as tile

    fdt = mybir.dt.float32
    bdt = mybir.dt.bfloat16
    KMAX = max(KT)

    nc = bacc.Bacc("TRN2", num_devices=N_CORES)
    xt1_in = nc.dram_tensor("xt1", [131, SHARD], bdt, kind="ExternalInput")
    w_ins = []
    for li, (cin, H, D) in enumerate(_LAYERS):
        kf = 2 if cin + 1 > 128 else 1
        w_ins.append(
            nc.dram_tensor(f"w{li+1}", [128, kf, 4 * H * D], bdt, kind="ExternalInput")
        )
    idx_ins, mask_ins = [], []
    for b, (t0, t1, Kb) in enumerate(BANDS):
        rows = (t1 - t0) * P
        idx_ins.append(
            nc.dram_tensor(f"idx{b}", [rows, Kb], mybir.dt.int32, kind="ExternalInput")
        )
        mask_ins.append(
            nc.dram_tensor(f"mask{b}", [rows, Kb], bdt, kind="ExternalInput")
        )
    band_of = []
    for t in range(NT):
        for b, (t0, t1, Kb) in enumerate(BANDS):
            if t0 <= t < t1:
                band_of.append(b)
                break
    y_out = nc.dram_tensor("y", [SHARD, 40], fdt, kind="ExternalOutput")

    with tile.TileContext(nc) as tc:
        with (
            tc.tile_pool(name="wpool", bufs=1) as wpool,
            tc.tile_pool(name="sb", bufs=2) as sb,
            tc.tile_pool(name="gp", bufs=2) as gp,
            tc.tile_pool(name="res", bufs=1) as res,
            tc.tile_pool(name="psum", bufs=4, space="PSUM") as pspool,
            tc.tile_pool(name="dram", bufs=1, space="DRAM") as dram,
        ):
            wts = []
            for li, (cin, H, D) in enumerate(_LAYERS):
                kf = 2 if cin + 1 > 128 else 1
                wt = wpool.tile([128, kf, 4 * H * D], bdt, tag=f"w{li}")
                nc.sync.dma_start(out=wt[:], in_=w_ins[li][:])
                wts.append(wt)

            h_prev = None  # DRAM [SHARD_PAD, F_prev + 1] bf16 (ones col last)
            for li, (cin, H, D) in enumerate(_LAYERS):
                F = H * D
                FQ = F // 2  # f32 words holding the bf16 q row
                ELEM = 2 * F
                cr = cin + 1
                kf = 2 if cr > 128 else 1
                m4 = 4 * F
                wt = wts[li]

                kv_local = dram.tile([SHARD, ELEM], bdt, tag=f"kvl{li}")
                kv_full = dram.tile([N_NODES, ELEM], bdt, tag=f"kvf{li}")
                qres = res.tile([P, NT * F], bdt, tag="qres")
                sres = res.tile([P, NT * F], fdt, tag="sres")
                h_tab = None
                if li < 2:
                    h_tab = dram.tile([SHARD_PAD, F + 1], bdt, tag=f"ht{li}")

                # -------- pass A: projections for own shard --------
                NCH = []
                c0 = 0
                while c0 < m4:
                    cn = min(400, m4 - c0)
                    NCH.append((c0, cn))
                    c0 += cn
                for t in range(NT):
                    m0 = t * P
                    m = min(P, SHARD - m0)
                    xt_t = sb.tile([128, kf, P], bdt, tag="xt")
                    nc.vector.memset(xt_t[:], 0)
                    if li == 0:
                        nc.sync.dma_start(
                            out=xt_t[:128, 0, :m], in_=xt1_in[0:128, m0 : m0 + m]
                        )
                        nc.sync.dma_start(
                            out=xt_t[0:3, 1, :m], in_=xt1_in[128:131, m0 : m0 + m]
                        )
                    else:
                        # h_prev has a ones column at index fp: the transpose
                        # loads features AND the bias ones-row together.
                        fp = _LAYERS[li - 1][1] * _LAYERS[li - 1][2]
                        r1 = min(fp + 1, 128)
                        nc.sync.dma_start_transpose(
                            out=xt_t[:r1, 0, :], in_=h_prev[m0 : m0 + P, 0:r1]
                        )
                        if fp + 1 > 128:
                            nc.sync.dma_start_transpose(
                                out=xt_t[: fp + 1 - 128, 1, :],
                                in_=h_prev[m0 : m0 + P, 128 : fp + 1],
                            )
                    kvb = sb.tile([P, ELEM], bdt, tag="kvb")
                    # W columns are ordered q|s|k|v so k|v is one contiguous copy
                    for (c0, cn) in NCH:
                        ps = pspool.tile([P, 400], fdt, tag="ps")
                        for ki in range(kf):
                            nc.tensor.matmul(
                                ps[:m, :cn],
                                lhsT=xt_t[:, ki, :m],
                                rhs=wt[:, ki, c0 : c0 + cn],
                                start=(ki == 0),
                                stop=(ki == kf - 1),
                            )
                        for dst_ap, soff, w_ in (
                            (qres[:m, t * F : (t + 1) * F], 0, F),
                            (sres[:m, t * F : (t + 1) * F], F, F),
                            (kvb[:m, :], 2 * F, 2 * F),
                        ):
                            lo = max(soff, c0)
                            hi = min(soff + w_, c0 + cn)
                            if lo < hi:
                                nc.vector.tensor_copy(
                                    out=dst_ap[:, lo - soff : hi - soff],
                                    in_=ps[:m, lo - c0 : hi - c0],
                                )
                    nc.sync.dma_start(out=kv_local[m0 : m0 + m, :], in_=kvb[:m, :])

                # -------- AllGather the k|v table --------
                nc.gpsimd.collective_compute(
                    "AllGather",
                    mybir.AluOpType.bypass,
                    replica_groups=[list(range(N_CORES))],
                    ins=[kv_local[:]],
                    outs=[kv_full[:]],
                )

                # -------- pass B: attention for own dst nodes --------
                for t in range(NT):
                    K = KT[t]
                    m0 = t * P
                    m = min(P, SHARD - m0)
                    b = band_of[t]
                    t0b = BANDS[b][0]
                    r0b = (t - t0b) * P
                    it = sb.tile([P, KMAX], mybir.dt.int32, tag="it")
                    nc.sync.dma_start(
                        out=it[:, 0:K], in_=idx_ins[b][r0b : r0b + P, 0:K]
                    )
                    mt = sb.tile([P, KMAX], bdt, tag="mt")
                    nc.sync.dma_start(
                        out=mt[:m, 0:K], in_=mask_ins[b][r0b : r0b + m, 0:K]
                    )
                    qt = qres[:, t * F : (t + 1) * F]
                    st = sres[:, t * F : (t + 1) * F]

                    g = gp.tile([P, KMAX, ELEM], bdt, tag="g")
                    for c in range(K):
                        nc.gpsimd.indirect_dma_start(
                            out=g[:, c, :],
                            out_offset=None,
                            in_=kv_full[:],
                            in_offset=bass.IndirectOffsetOnAxis(
                                ap=it[:, c : c + 1], axis=0
                            ),
                        )
                    prod = gp.tile([P, KMAX, F], bdt, tag="pw")
                    nc.vector.tensor_tensor(
                        out=prod[:m, 0:K, :],
                        in0=g[:m, 0:K, 0:F],
                        in1=qt[:m, :].unsqueeze(1).broadcast_to([m, K, F]),
                        op=mybir.AluOpType.mult,
                    )
                    scores = sb.tile([P, KMAX * H], fdt, tag="scores")
                    nc.vector.tensor_reduce(
                        out=scores[:m, 0 : K * H],
                        in_=prod[:m, 0:K, :].rearrange("p k (h d) -> p (k h) d", h=H),
                        axis=mybir.AxisListType.X,
                        op=mybir.AluOpType.add,
                    )
                    sm = sb.tile([P, KMAX * H], fdt, tag="sm")
                    nc.vector.scalar_tensor_tensor(
                        out=sm[:m, 0 : K * H].rearrange("p (k h) -> p k h", k=K),
                        in0=scores[:m, 0 : K * H].rearrange("p (k h) -> p k h", k=K),
                        scalar=60.0,
                        in1=mt[:m, 0:K].to_broadcast([m, K, H]),
                        op0=mybir.AluOpType.min,
                        op1=mybir.AluOpType.add,
                    )
                    es = sb.tile([P, KMAX * H], bdt, tag="es")
                    nc.scalar.activation(
                        out=es[:m, 0 : K * H],
                        in_=sm[:m, 0 : K * H],
                        func=mybir.ActivationFunctionType.Exp,
                    )
                    dn = sb.tile([P, H], fdt, tag="dn")
                    nc.vector.tensor_reduce(
                        out=dn[:m],
                        in_=es[:m, 0 : K * H].rearrange("p (k h) -> p h k", k=K),
                        axis=mybir.AxisListType.X,
                        op=mybir.AluOpType.add,
                    )
                    wv = gp.tile([P, KMAX, F], bdt, tag="pw")
                    nc.vector.tensor_tensor(
                        out=wv[:m, 0:K, :].rearrange("p k (h d) -> p k h d", h=H),
                        in0=g[:m, 0:K, F:ELEM].rearrange("p k (h d) -> p k h d", h=H),
                        in1=es[:m, 0 : K * H]
                        .rearrange("p (k h) -> p k h", k=K)
                        .unsqueeze(3)
                        .broadcast_to([m, K, H, D]),
                        op=mybir.AluOpType.mult,
                    )
                    osum = sb.tile([P, F], fdt, tag="osum")
                    nc.vector.tensor_reduce(
                        out=osum[:m],
                        in_=wv[:m, 0:K, :].rearrange("p k f -> p f k"),
                        axis=mybir.AxisListType.X,
                        op=mybir.AluOpType.add,
                    )
                    rec = sb.tile([P, H], fdt, tag="rec")
                    nc.vector.tensor_scalar_add(out=rec[:m], in0=dn[:m], scalar1=1e-16)
                    nc.vector.reciprocal(out=rec[:m], in_=rec[:m])
                    hsb = sb.tile([P, F], fdt, tag="hsb")
                    nc.vector.tensor_tensor(
                        out=hsb[:m].rearrange("p (h d) -> p h d", h=H),
                        in0=osum[:m].rearrange("p (h d) -> p h d", h=H),
                        in1=rec[:m].unsqueeze(2).broadcast_to([m, H, D]),
                        op=mybir.AluOpType.mult,
                    )
                    nc.vector.tensor_tensor(
                        out=hsb[:m], in0=hsb[:m], in1=st[:m, :], op=mybir.AluOpType.add
                    )
                    if li < 2:
                        hb = sb.tile([P, F + 1], bdt, tag="hb")
                        # LeakyReLU(x) = max(x, 0.1*x)
                        nc.vector.scalar_tensor_tensor(
                            out=hb[:m, 0:F],
                            in0=hsb[:m],
                            scalar=LEAKY_ALPHA,
                            in1=hsb[:m],
                            op0=mybir.AluOpType.mult,
                            op1=mybir.AluOpType.max,
                        )
                        nc.vector.memset(hb[:m, F : F + 1], 1.0)
                        nc.sync.dma_start(out=h_tab[m0 : m0 + m, :], in_=hb[:m, :])
                    else:
                        # log_softmax over the 40 output columns
                        negm = sb.tile([P, 1], fdt, tag="negm")
                        nc.vector.tensor_reduce(
                            out=negm[:m],
                            in_=hsb[:m],
                            axis=mybir.AxisListType.X,
                            op=mybir.AluOpType.max,
                            negate=True,
                        )
                        z = sb.tile([P, F], fdt, tag="z")
                        nc.scalar.activation(
                            out=z[:m],
                            in_=hsb[:m],
                            func=mybir.ActivationFunctionType.Identity,
                            bias=negm[:m],
                        )
                        ez = sb.tile([P, F], fdt, tag="ez")
                        se = sb.tile([P, 1], fdt, tag="se")
                        nc.scalar.activation(
                            out=ez[:m],
                            in_=z[:m],
                            func=mybir.ActivationFunctionType.Exp,
                            accum_out=se[:m],
                        )
                        ls = sb.tile([P, 1], fdt, tag="ls")
                        nc.scalar.activation(
                            out=ls[:m], in_=se[:m], func=mybir.ActivationFunctionType.Ln
                        )
                        out_t = sb.tile([P, F], fdt, tag="out")
                        nc.vector.tensor_tensor(
                            out=out_t[:m],
                            in0=z[:m],
                            in1=ls[:m].broadcast_to([m, F]),
                            op=mybir.AluOpType.subtract,
                        )
                        nc.sync.dma_start(out=y_out[m0 : m0 + m, :], in_=out_t[:m, :])
                h_prev = h_tab
    nc.compile()
    return nc


def _prep_structure(src, dst):
    """Degree-sorted per-core slot tables.

    Returns (banded idx/mask per core, KT, bands, perm) where
    perm maps new (degree-sorted) global node id -> old global node id.
    """
    deg = np.bincount(dst, minlength=N_NODES)
    perm = np.empty(N_NODES, np.int64)
    for c in range(N_CORES):
        sl = slice(c * SHARD, (c + 1) * SHARD)
        order_c = np.argsort(-deg[sl], kind="stable")
        perm[sl] = c * SHARD + order_c
    inv_perm = np.empty(N_NODES, np.int64)
    inv_perm[perm] = np.arange(N_NODES)

    ndst = inv_perm[dst]
    nsrc = inv_perm[src]
    order = np.argsort(ndst, kind="stable")
    dsorted = ndst[order]
    ssorted = nsrc[order]
    ndeg = np.bincount(dsorted, minlength=N_NODES)
    starts = np.zeros(N_NODES + 1, np.int64)
    np.cumsum(ndeg, out=starts[1:])
    rank = np.arange(dsorted.shape[0], dtype=np.int64) - starts[dsorted]

    dmat = ndeg.reshape(N_CORES, SHARD)
    KT = []
    for t in range(NT):
        hi = min((t + 1) * P, SHARD)
        KT.append(max(1, int(dmat[:, t * P : hi].max())))

    KMAX = max(KT)
    idx = np.zeros((N_NODES, KMAX), np.int32)
    maskb = np.full((N_NODES, KMAX), -30000.0, np.float32)
    idx[dsorted, rank] = ssorted.astype(np.int32)
    maskb[dsorted, rank] = 0.0

    # width bands over the (non-increasing) KT: 4 bands minimizing padded area
    nb = 4
    INF = 1 << 60
    cost = [[INF] * (nb + 1) for _ in range(NT + 1)]
    prevb = [[-1] * (nb + 1) for _ in range(NT + 1)]
    cost[0][0] = 0
    for t1 in range(1, NT + 1):
        for b in range(1, nb + 1):
            for t0 in range(t1):
                if cost[t0][b - 1] == INF:
                    continue
                w = KT[t0] * (t1 - t0)  # KT non-increasing: band width = KT[t0]
                if cost[t0][b - 1] + w < cost[t1][b]:
                    cost[t1][b] = cost[t0][b - 1] + w
                    prevb[t1][b] = t0
    bands = []
    t1, b = NT, nb
    while t1 > 0:
        t0 = prevb[t1][b]
        bands.append((t0, t1, KT[t0]))
        t1, b = t0, b - 1
    bands.reverse()

    import ml_dtypes

    maskh = maskb.astype(ml_dtypes.bfloat16)
    idx_bc = []   # idx_bc[c][b], mask_bc[c][b]
    mask_bc = []
    for c in range(N_CORES):
        ib_list, mb_list = [], []
        for (t0, t1, Kb) in bands:
            rows = (t1 - t0) * P
            lo = c * SHARD + t0 * P
            hi = min(c * SHARD + t1 * P, (c + 1) * SHARD)
            blk_i = np.zeros((rows, Kb), np.int32)
            blk_m = np.full((rows, Kb), -30000.0, np.float32).astype(ml_dtypes.bfloat16)
            blk_i[: hi - lo] = idx[lo:hi, :Kb]
            blk_m[: hi - lo] = maskh[lo:hi, :Kb]
            ib_list.append(np.ascontiguousarray(blk_i))
            mb_list.append(np.ascontiguousarray(blk_m))
        idx_bc.append(ib_list)
        mask_bc.append(mb_list)
    return idx_bc, mask_bc, KT, bands, perm


def _fold_w(W4, b4, cin, scale_q, F):
    # W4/b4 arrive in q|s|k|v column order; scale applies to the q block
    import ml_dtypes

    kf = 2 if cin + 1 > 128 else 1
    w = np.zeros((128 * kf, 4 * F), np.float32)
    w[:cin] = W4
    w[cin] = b4
    w[:, 0:F] *= scale_q
    return (
        w.astype(ml_dtypes.bfloat16).reshape(kf, 128, 4 * F).transpose(1, 0, 2).copy()
    )


def _get_program(KT, BANDS):
    key = (tuple(KT), tuple(BANDS))
    if _STATE.get("key") != key:
        nc = _build_program(KT, BANDS)
        import ml_dtypes
        from concourse import bass2jax

        dummy = []
        for _ in range(N_CORES):
            d = {
                "xt1": np.zeros((131, SHARD), ml_dtypes.bfloat16),
            }
            for b, (t0, t1, Kb) in enumerate(BANDS):
                rows = (t1 - t0) * P
                d[f"idx{b}"] = np.zeros((rows, Kb), np.int32)
                d[f"mask{b}"] = np.full(
                    (rows, Kb), -30000.0, np.float32
                ).astype(ml_dtypes.bfloat16)
            for li, (cin, H, D) in enumerate(_LAYERS):
                kf = 2 if cin + 1 > 128 else 1
                d[f"w{li+1}"] = np.zeros((128, kf, 4 * H * D), ml_dtypes.bfloat16)
            dummy.append(d)
        bass2jax.run_bass_via_pjrt(nc, dummy, n_cores=N_CORES)
        _STATE["key"] = key
        _STATE["nc"] = nc
    return _STATE["nc"]


def kernel(**inputs):
    import ml_dtypes
    from concourse.bass_utils import run_bass_kernel_spmd

    x = np.asarray(inputs["x"], np.float32)
    edge_index = np.asarray(inputs["edge_index"])
    src = edge_index[0].astype(np.int64)
    dst = edge_index[1].astype(np.int64)

    idx_bc, mask_bc, KT, bands, perm = _prep_structure(src, dst)
    nc = _get_program(KT, bands)

    ws = []
    for li, (cin, H, D) in enumerate(_LAYERS):
        W4 = np.concatenate(
            [
                np.asarray(inputs[f"W{nm}{li+1}"], np.float32)
                for nm in ["q", "s", "k", "v"]
            ],
            axis=1,
        )
        b4 = np.concatenate(
            [
                np.asarray(inputs[f"b{nm}{li+1}"], np.float32)
                for nm in ["q", "s", "k", "v"]
            ]
        )
        ws.append(_fold_w(W4, b4, cin, 1.0 / np.sqrt(np.float32(D)), H * D))

    xp = x[perm]  # rows in new (degree-sorted) order
    xb = xp.astype(ml_dtypes.bfloat16)
    ones = np.ones((1, SHARD), ml_dtypes.bfloat16)
    in_maps = []
    for c in range(N_CORES):
        sl = slice(c * SHARD, (c + 1) * SHARD)
        xt1 = np.ascontiguousarray(np.concatenate([xb[sl].T, ones], axis=0))
        m = {"xt1": xt1}
        for b in range(len(bands)):
            m[f"idx{b}"] = idx_bc[c][b]
            m[f"mask{b}"] = mask_bc[c][b]
        for li in range(3):
            m[f"w{li+1}"] = ws[li]
        in_maps.append(m)

    import time as _time

    t0 = _time.time()
    res = run_bass_kernel_spmd(nc, in_maps, list(range(N_CORES)))
    dt = int((_time.time() - t0) * 1e9)
    globals()["_DEVICE_WALL_NS"] = globals().get("_DEVICE_WALL_NS", 0) + dt
    globals().setdefault("_LAUNCH_NS", []).append(dt)

    y_perm = np.concatenate(
        [res.results[c]["y"].astype(np.float32) for c in range(N_CORES)], axis=0
    )
    y = np.empty_like(y_perm)
    y[perm] = y_perm  # un-permute rows back to original node order
    return y


# revision 7
# speedup vs baseline: 4.7267x; 1.0715x over previous
"""Trainium2 one-launch kernel for nn_Net_68994354643186 (3-layer
TransformerConv GNN).

Everything runs on the 8 NeuronCores in a SINGLE SPMD launch:
  - Nodes are partitioned into 8 contiguous shards of 6250 (edge-cut),
    and each core's shard is re-ordered by in-degree so the slot-padded
    edge tables are tight per 128-node tile (per-tile slot count K_t is
    specialized into the program at build time).
  - Per layer, each core computes the fused q|k|v|s projection GEMM for
    its shard (bf16, bias folded via ones-row, 1/sqrt(D) folded into Wq),
    writes a packed [k|v] bf16 row table, AllGathers it so every core
    holds the full 50000-row k|v table, then runs the per-edge attention
    for its own destination nodes: indirect-DMA row gathers (128 rows
    per instruction, one per dest-node partition), masked exp-softmax
    without max-subtraction (scores clamped at 60), strided slot-sum
    reductions, the root/skip add and LeakyReLU.  Layer outputs stay
    on-device (bf16, with a ones column for the next bias) and are
    re-loaded transposed via DMA-transpose as the next layer's GEMM
    input.
  - The host only builds the degree-sorted slot tables, uploads
    x^T / weights / packed slot indices+mask, and un-permutes the
    concatenated output shards.

The program is compiled and executed once for warm-up (AOT) before the
timed launch, so the timed launch is pure transfer + execute.

Self-contained: hardcodes all shapes; no sibling imports.
"""

import sys

sys.path.insert(0, "/opt/trn_rl_repo")

import numpy as np

N_NODES = 50000
N_EDGES = 800000
N_CORES = 8
SHARD = N_NODES // N_CORES  # 6250
LEAKY_ALPHA = 0.1
P = 128
NT = (SHARD + P - 1) // P  # 49 tiles; last has 106 real rows
SHARD_PAD = NT * P         # 6272

_LAYERS = [
    # (cin, heads, head_dim)
    (130, 4, 50),
    (200, 4, 25),
    (100, 4, 10),
]

_STATE = {}


def _build_program(KT, BANDS):
    """KT: per-tile slot counts (len NT); BANDS: [(t0, t1, Kb)] width bands."""
    import concourse.bass as bass
    import concourse.bacc as bacc
    import concourse.mybir as mybir
    import concourse.tile as tile

    fdt = mybir.dt.float32
    bdt = mybir.dt.bfloat16
    KMAX = max(KT)

    nc = bacc.Bacc("TRN2", num_devices=N_CORES)
    xt1_in = nc.dram_tensor("xt1", [131, SHARD], bdt, kind="ExternalInput")
    w_ins = []
    for li, (cin, H, D) in enumerate(_LAYERS):
        kf = 2 if cin + 1 > 128 else 1
        w_ins.append(
            nc.dram_tensor(f"w{li+1}", [128, kf, 4 * H * D], bdt, kind="ExternalInput")
        )
    idx_ins, mask_ins = [], []
    for b, (t0, t1, Kb) in enumerate(BANDS):
        rows = (t1 - t0) * P
        idx_ins.append(
            nc.dram_tensor(f"idx{b}", [rows, Kb], mybir.dt.int32, kind="ExternalInput")
        )
        mask_ins.append(
            nc.dram_tensor(f"mask{b}", [rows, Kb], bdt, kind="ExternalInput")
        )
    band_of = []
    for t in range(NT):
        for b, (t0, t1, Kb) in enumerate(BANDS):
            if t0 <= t < t1:
                band_of.append(b)
                break
    y_out = nc.dram_tensor("y", [SHARD, 40], bdt, kind="ExternalOutput")

    with tile.TileContext(nc) as tc:
        with (
            tc.tile_pool(name="wpool", bufs=1) as wpool,
            tc.tile_pool(name="sb", bufs=2) as sb,
            tc.tile_pool(name="gp", bufs=2) as gp,
            tc.tile_pool(name="res", bufs=1) as res,
            tc.tile_pool(name="psum", bufs=4, space="PSUM") as pspool,
            tc.tile_pool(name="dram", bufs=1, space="DRAM") as dram,
        ):
            wts = []
            for li, (cin, H, D) in enumerate(_LAYERS):
                kf = 2 if cin + 1 > 128 else 1
                wt = wpool.tile([128, kf, 4 * H * D], bdt, tag=f"w{li}")
                nc.sync.dma_start(out=wt[:], in_=w_ins[li][:])
                wts.append(wt)

            h_prev = None  # DRAM [SHARD_PAD, F_prev + 1] bf16 (ones col last)
            for li, (cin, H, D) in enumerate(_LAYERS):
                F = H * D
                FQ = F // 2  # f32 words holding the bf16 q row
                ELEM = 2 * F
                cr = cin + 1
                kf = 2 if cr > 128 else 1
                m4 = 4 * F
                wt = wts[li]

                kv_local = dram.tile([SHARD, ELEM], bdt, tag=f"kvl{li}")
                kv_full = dram.tile([N_NODES, ELEM], bdt, tag=f"kvf{li}")
                qres = res.tile([P, NT * F], bdt, tag="qres")
                sres = res.tile([P, NT * F], fdt, tag="sres")
                h_tab = None
                if li < 2:
                    h_tab = dram.tile([SHARD_PAD, F + 1], bdt, tag=f"ht{li}")

                # -------- pass A: projections for own shard --------
                NCH = []
                c0 = 0
                while c0 < m4:
                    cn = min(400, m4 - c0)
                    NCH.append((c0, cn))
                    c0 += cn
                for t in range(NT):
                    m0 = t * P
                    m = min(P, SHARD - m0)
                    xt_t = sb.tile([128, kf, P], bdt, tag="xt")
                    nc.vector.memset(xt_t[:], 0)
                    if li == 0:
                        nc.sync.dma_start(
                            out=xt_t[:128, 0, :m], in_=xt1_in[0:128, m0 : m0 + m]
                        )
                        nc.sync.dma_start(
                            out=xt_t[0:3, 1, :m], in_=xt1_in[128:131, m0 : m0 + m]
                        )
                    else:
                        # h_prev has a ones column at index fp: the transpose
                        # loads features AND the bias ones-row together.
                        fp = _LAYERS[li - 1][1] * _LAYERS[li - 1][2]
                        r1 = min(fp + 1, 128)
                        nc.sync.dma_start_transpose(
                            out=xt_t[:r1, 0, :], in_=h_prev[m0 : m0 + P, 0:r1]
                        )
                        if fp + 1 > 128:
                            nc.sync.dma_start_transpose(
                                out=xt_t[: fp + 1 - 128, 1, :],
                                in_=h_prev[m0 : m0 + P, 128 : fp + 1],
                            )
                    kvb = sb.tile([P, ELEM], bdt, tag="kvb")
                    # W columns are ordered q|s|k|v so k|v is one contiguous copy
                    for (c0, cn) in NCH:
                        ps = pspool.tile([P, 400], fdt, tag="ps")
                        for ki in range(kf):
                            nc.tensor.matmul(
                                ps[:m, :cn],
                                lhsT=xt_t[:, ki, :m],
                                rhs=wt[:, ki, c0 : c0 + cn],
                                start=(ki == 0),
                                stop=(ki == kf - 1),
                            )
                        for dst_ap, soff, w_ in (
                            (qres[:m, t * F : (t + 1) * F], 0, F),
                            (sres[:m, t * F : (t + 1) * F], F, F),
                            (kvb[:m, :], 2 * F, 2 * F),
                        ):
                            lo = max(soff, c0)
                            hi = min(soff + w_, c0 + cn)
                            if lo < hi:
                                nc.vector.tensor_copy(
                                    out=dst_ap[:, lo - soff : hi - soff],
                                    in_=ps[:m, lo - c0 : hi - c0],
                                )
                    nc.sync.dma_start(out=kv_local[m0 : m0 + m, :], in_=kvb[:m, :])

                # -------- AllGather the k|v table --------
                nc.gpsimd.collective_compute(
                    "AllGather",
                    mybir.AluOpType.bypass,
                    replica_groups=[list(range(N_CORES))],
                    ins=[kv_local[:]],
                    outs=[kv_full[:]],
                )

                # -------- pass B: attention for own dst nodes --------
                for t in range(NT):
                    K = KT[t]
                    m0 = t * P
                    m = min(P, SHARD - m0)
                    b = band_of[t]
                    t0b = BANDS[b][0]
                    r0b = (t - t0b) * P
                    it = sb.tile([P, KMAX], mybir.dt.int32, tag="it")
                    nc.sync.dma_start(
                        out=it[:, 0:K], in_=idx_ins[b][r0b : r0b + P, 0:K]
                    )
                    mt = sb.tile([P, KMAX], bdt, tag="mt")
                    nc.sync.dma_start(
                        out=mt[:m, 0:K], in_=mask_ins[b][r0b : r0b + m, 0:K]
                    )
                    qt = qres[:, t * F : (t + 1) * F]
                    st = sres[:, t * F : (t + 1) * F]

                    g = gp.tile([P, KMAX, ELEM], bdt, tag="g")
                    for c in range(K):
                        nc.gpsimd.indirect_dma_start(
                            out=g[:, c, :],
                            out_offset=None,
                            in_=kv_full[:],
                            in_offset=bass.IndirectOffsetOnAxis(
                                ap=it[:, c : c + 1], axis=0
                            ),
                        )
                    prod = gp.tile([P, KMAX, F], bdt, tag="pw")
                    nc.vector.tensor_tensor(
                        out=prod[:m, 0:K, :],
                        in0=g[:m, 0:K, 0:F],
                        in1=qt[:m, :].unsqueeze(1).broadcast_to([m, K, F]),
                        op=mybir.AluOpType.mult,
                    )
                    scores = sb.tile([P, KMAX * H], fdt, tag="scores")
                    nc.vector.tensor_reduce(
                        out=scores[:m, 0 : K * H],
                        in_=prod[:m, 0:K, :].rearrange("p k (h d) -> p (k h) d", h=H),
                        axis=mybir.AxisListType.X,
                        op=mybir.AluOpType.add,
                    )
                    sm = sb.tile([P, KMAX * H], fdt, tag="sm")
                    nc.vector.scalar_tensor_tensor(
                        out=sm[:m, 0 : K * H].rearrange("p (k h) -> p k h", k=K),
                        in0=scores[:m, 0 : K * H].rearrange("p (k h) -> p k h", k=K),
                        scalar=60.0,
                        in1=mt[:m, 0:K].to_broadcast([m, K, H]),
                        op0=mybir.AluOpType.min,
                        op1=mybir.AluOpType.add,
                    )
                    es = sb.tile([P, KMAX * H], bdt, tag="es")
                    nc.scalar.activation(
                        out=es[:m, 0 : K * H],
                        in_=sm[:m, 0 : K * H],
                        func=mybir.ActivationFunctionType.Exp,
                    )
                    dn = sb.tile([P, H], fdt, tag="dn")
                    nc.vector.tensor_reduce(
                        out=dn[:m],
                        in_=es[:m, 0 : K * H].rearrange("p (k h) -> p h k", k=K),
                        axis=mybir.AxisListType.X,
                        op=mybir.AluOpType.add,
                    )
                    wv = gp.tile([P, KMAX, F], bdt, tag="pw")
                    nc.vector.tensor_tensor(
                        out=wv[:m, 0:K, :].rearrange("p k (h d) -> p k h d", h=H),
                        in0=g[:m, 0:K, F:ELEM].rearrange("p k (h d) -> p k h d", h=H),
                        in1=es[:m, 0 : K * H]
                        .rearrange("p (k h) -> p k h", k=K)
                        .unsqueeze(3)
                        .broadcast_to([m, K, H, D]),
                        op=mybir.AluOpType.mult,
                    )
                    osum = sb.tile([P, F], fdt, tag="osum")
                    nc.vector.tensor_reduce(
                        out=osum[:m],
                        in_=wv[:m, 0:K, :].rearrange("p k f -> p f k"),
                        axis=mybir.AxisListType.X,
                        op=mybir.AluOpType.add,
                    )
                    rec = sb.tile([P, H], fdt, tag="rec")
                    nc.vector.reciprocal(out=rec[:m], in_=dn[:m])
                    hsb = sb.tile([P, F], fdt, tag="hsb")
                    nc.vector.tensor_tensor(
                        out=hsb[:m].rearrange("p (h d) -> p h d", h=H),
                        in0=osum[:m].rearrange("p (h d) -> p h d", h=H),
                        in1=rec[:m].unsqueeze(2).broadcast_to([m, H, D]),
                        op=mybir.AluOpType.mult,
                    )
                    nc.vector.tensor_tensor(
                        out=hsb[:m], in0=hsb[:m], in1=st[:m, :], op=mybir.AluOpType.add
                    )
                    if li < 2:
                        hb = sb.tile([P, F + 1], bdt, tag="hb")
                        # LeakyReLU(x) = max(x, 0.1*x)
                        nc.vector.scalar_tensor_tensor(
                            out=hb[:m, 0:F],
                            in0=hsb[:m],
                            scalar=LEAKY_ALPHA,
                            in1=hsb[:m],
                            op0=mybir.AluOpType.mult,
                            op1=mybir.AluOpType.max,
                        )
                        nc.vector.memset(hb[:m, F : F + 1], 1.0)
                        nc.sync.dma_start(out=h_tab[m0 : m0 + m, :], in_=hb[:m, :])
                    else:
                        # log_softmax over the 40 output columns
                        negm = sb.tile([P, 1], fdt, tag="negm")
                        nc.vector.tensor_reduce(
                            out=negm[:m],
                            in_=hsb[:m],
                            axis=mybir.AxisListType.X,
                            op=mybir.AluOpType.max,
                            negate=True,
                        )
                        z = sb.tile([P, F], fdt, tag="z")
                        nc.scalar.activation(
                            out=z[:m],
                            in_=hsb[:m],
                            func=mybir.ActivationFunctionType.Identity,
                            bias=negm[:m],
                        )
                        ez = sb.tile([P, F], fdt, tag="ez")
                        se = sb.tile([P, 1], fdt, tag="se")
                        nc.scalar.activation(
                            out=ez[:m],
                            in_=z[:m],
                            func=mybir.ActivationFunctionType.Exp,
                            accum_out=se[:m],
                        )
                        ls = sb.tile([P, 1], fdt, tag="ls")
                        nc.scalar.activation(
                            out=ls[:m], in_=se[:m], func=mybir.ActivationFunctionType.Ln
                        )
                        out_t = sb.tile([P, F], bdt, tag="out")
                        nc.vector.tensor_tensor(
                            out=out_t[:m],
                            in0=z[:m],
                            in1=ls[:m].broadcast_to([m, F]),
                            op=mybir.AluOpType.subtract,
                        )
                        nc.sync.dma_start(out=y_out[m0 : m0 + m, :], in_=out_t[:m, :])
                h_prev = h_tab
    nc.compile()
    return nc


def _prep_structure(src, dst):
    """Degree-sorted per-core slot tables.

    Returns (banded idx/mask per core, KT, bands, perm) where
    perm maps new (degree-sorted) global node id -> old global node id.
    """
    deg = np.bincount(dst, minlength=N_NODES)
    assert deg.min() >= 1, "zero in-degree node: reciprocal needs the epsilon path"
    perm = np.empty(N_NODES, np.int64)
    for c in range(N_CORES):
        sl = slice(c * SHARD, (c + 1) * SHARD)
        order_c = np.argsort(-deg[sl], kind="stable")
        perm[sl] = c * SHARD + order_c
    inv_perm = np.empty(N_NODES, np.int64)
    inv_perm[perm] = np.arange(N_NODES)

    ndst = inv_perm[dst]
    nsrc = inv_perm[src]
    order = np.argsort(ndst, kind="stable")
    dsorted = ndst[order]
    ssorted = nsrc[order]
    ndeg = np.bincount(dsorted, minlength=N_NODES)
    starts = np.zeros(N_NODES + 1, np.int64)
    np.cumsum(ndeg, out=starts[1:])
    rank = np.arange(dsorted.shape[0], dtype=np.int64) - starts[dsorted]

    dmat = ndeg.reshape(N_CORES, SHARD)
    KT = []
    for t in range(NT):
        hi = min((t + 1) * P, SHARD)
        KT.append(max(1, int(dmat[:, t * P : hi].max())))

    KMAX = max(KT)
    idx = np.zeros((N_NODES, KMAX), np.int32)
    maskb = np.full((N_NODES, KMAX), -30000.0, np.float32)
    idx[dsorted, rank] = ssorted.astype(np.int32)
    maskb[dsorted, rank] = 0.0

    # width bands over the (non-increasing) KT: 4 bands minimizing padded area
    nb = 4
    INF = 1 << 60
    cost = [[INF] * (nb + 1) for _ in range(NT + 1)]
    prevb = [[-1] * (nb + 1) for _ in range(NT + 1)]
    cost[0][0] = 0
    for t1 in range(1, NT + 1):
        for b in range(1, nb + 1):
            for t0 in range(t1):
                if cost[t0][b - 1] == INF:
                    continue
                w = KT[t0] * (t1 - t0)  # KT non-increasing: band width = KT[t0]
                if cost[t0][b - 1] + w < cost[t1][b]:
                    cost[t1][b] = cost[t0][b - 1] + w
                    prevb[t1][b] = t0
    bands = []
    t1, b = NT, nb
    while t1 > 0:
        t0 = prevb[t1][b]
        bands.append((t0, t1, KT[t0]))
        t1, b = t0, b - 1
    bands.reverse()

    import ml_dtypes

    maskh = maskb.astype(ml_dtypes.bfloat16)
    idx_bc = []   # idx_bc[c][b], mask_bc[c][b]
    mask_bc = []
    for c in range(N_CORES):
        ib_list, mb_list = [], []
        for (t0, t1, Kb) in bands:
            rows = (t1 - t0) * P
            lo = c * SHARD + t0 * P
            hi = min(c * SHARD + t1 * P, (c + 1) * SHARD)
            blk_i = np.zeros((rows, Kb), np.int32)
            blk_m = np.full((rows, Kb), -30000.0, np.float32).astype(ml_dtypes.bfloat16)
            blk_i[: hi - lo] = idx[lo:hi, :Kb]
            blk_m[: hi - lo] = maskh[lo:hi, :Kb]
            ib_list.append(np.ascontiguousarray(blk_i))
            mb_list.append(np.ascontiguousarray(blk_m))
        idx_bc.append(ib_list)
        mask_bc.append(mb_list)
    return idx_bc, mask_bc, KT, bands, perm


def _fold_w(W4, b4, cin, scale_q, F):
    # W4/b4 arrive in q|s|k|v column order; scale applies to the q block
    import ml_dtypes

    kf = 2 if cin + 1 > 128 else 1
    w = np.zeros((128 * kf, 4 * F), np.float32)
    w[:cin] = W4
    w[cin] = b4
    w[:, 0:F] *= scale_q
    return (
        w.astype(ml_dtypes.bfloat16).reshape(kf, 128, 4 * F).transpose(1, 0, 2).copy()
    )


def _get_program(KT, BANDS):
    key = (tuple(KT), tuple(BANDS))
    if _STATE.get("key") != key:
        nc = _build_program(KT, BANDS)
        import ml_dtypes
        from concourse import bass2jax

        dummy = []
        for _ in range(N_CORES):
            d = {
                "xt1": np.zeros((131, SHARD), ml_dtypes.bfloat16),
            }
            for b, (t0, t1, Kb) in enumerate(BANDS):
                rows = (t1 - t0) * P
                d[f"idx{b}"] = np.zeros((rows, Kb), np.int32)
                d[f"mask{b}"] = np.full(
                    (rows, Kb), -30000.0, np.float32
                ).astype(ml_dtypes.bfloat16)
            for li, (cin, H, D) in enumerate(_LAYERS):
                kf = 2 if cin + 1 > 128 else 1
                d[f"w{li+1}"] = np.zeros((128, kf, 4 * H * D), ml_dtypes.bfloat16)
            dummy.append(d)
        bass2jax.run_bass_via_pjrt(nc, dummy, n_cores=N_CORES)
        _STATE["key"] = key
        _STATE["nc"] = nc
    return _STATE["nc"]


def kernel(**inputs):
    import ml_dtypes
    from concourse.bass_utils import run_bass_kernel_spmd

    x = np.asarray(inputs["x"], np.float32)
    edge_index = np.asarray(inputs["edge_index"])
    src = edge_index[0].astype(np.int64)
    dst = edge_index[1].astype(np.int64)

    idx_bc, mask_bc, KT, bands, perm = _prep_structure(src, dst)
    nc = _get_program(KT, bands)

    ws = []
    for li, (cin, H, D) in enumerate(_LAYERS):
        W4 = np.concatenate(
            [
                np.asarray(inputs[f"W{nm}{li+1}"], np.float32)
                for nm in ["q", "s", "k", "v"]
            ],
            axis=1,
        )
        b4 = np.concatenate(
            [
                np.asarray(inputs[f"b{nm}{li+1}"], np.float32)
                for nm in ["q", "s", "k", "v"]
            ]
        )
        ws.append(_fold_w(W4, b4, cin, 1.0 / np.sqrt(np.float32(D)), H * D))

    xp = x[perm]  # rows in new (degree-sorted) order
    xb = xp.astype(ml_dtypes.bfloat16)
    ones = np.ones((1, SHARD), ml_dtypes.bfloat16)
    in_maps = []
    for c in range(N_CORES):
        sl = slice(c * SHARD, (c + 1) * SHARD)
        xt1 = np.ascontiguousarray(np.concatenate([xb[sl].T, ones], axis=0))
        m = {"xt1": xt1}
        for b in range(len(bands)):
            m[f"idx{b}"] = idx_bc[c][b]
            m[f"mask{b}"] = mask_bc[c][b]
        for li in range(3):
            m[f"w{li+1}"] = ws[li]
        in_maps.append(m)

    import time as _time

    t0 = _time.time()
    res = run_bass_kernel_spmd(nc, in_maps, list(range(N_CORES)))
    dt = int((_time.time() - t0) * 1e9)
    globals()["_DEVICE_WALL_NS"] = globals().get("_DEVICE_WALL_NS", 0) + dt
    globals().setdefault("_LAUNCH_NS", []).append(dt)

    y_perm = np.concatenate(
        [res.results[c]["y"].astype(np.float32) for c in range(N_CORES)], axis=0
    )
    y = np.empty_like(y_perm)
    y[perm] = y_perm  # un-permute rows back to original node order
    return y


# revision 8
# speedup vs baseline: 7.4652x; 1.5794x over previous
"""Trainium2 one-launch kernel for nn_Net_68994354643186 (3-layer
TransformerConv GNN).

Everything runs on the 8 NeuronCores in a SINGLE SPMD launch:
  - Nodes are partitioned into 8 contiguous shards of 6250 (edge-cut),
    and each core's shard is re-ordered by in-degree so the slot-padded
    edge tables are tight per 128-node tile (per-tile slot count K_t is
    specialized into the program at build time).
  - Per layer, each core computes the fused q|k|v|s projection GEMM for
    its shard (bf16, bias folded via ones-row, 1/sqrt(D) folded into Wq),
    writes a packed [k|v] bf16 row table, AllGathers it so every core
    holds the full 50000-row k|v table, then runs the per-edge attention
    for its own destination nodes: indirect-DMA row gathers (128 rows
    per instruction, one per dest-node partition), masked exp-softmax
    without max-subtraction (scores clamped at 60), strided slot-sum
    reductions, the root/skip add and LeakyReLU.  Layer outputs stay
    on-device (bf16, with a ones column for the next bias) and are
    re-loaded transposed via DMA-transpose as the next layer's GEMM
    input.
  - The host only builds the degree-sorted slot tables, uploads
    x^T / weights / packed slot indices+mask, and un-permutes the
    concatenated output shards.

The program is compiled and executed once for warm-up (AOT) before the
timed launch, so the timed launch is pure transfer + execute.

Self-contained: hardcodes all shapes; no sibling imports.
"""

import sys

sys.path.insert(0, "/opt/trn_rl_repo")

import numpy as np

N_NODES = 50000
N_EDGES = 800000
N_CORES = 8
SHARD = N_NODES // N_CORES  # 6250
LEAKY_ALPHA = 0.1
P = 128
NT = (SHARD + P - 1) // P  # 49 tiles; last has 106 real rows
SHARD_PAD = NT * P         # 6272

_LAYERS = [
    # (cin, heads, head_dim)
    (130, 4, 50),
    (200, 4, 25),
    (100, 4, 10),
]

_STATE = {}


def _build_program(KT, BANDS, BATCHES):
    """KT: per-tile slot counts; BANDS: [(t0,t1,Kb)]; BATCHES: [(t0,G,b)]."""
    import concourse.bass as bass
    import concourse.bacc as bacc
    import concourse.mybir as mybir
    import concourse.tile as tile

    fdt = mybir.dt.float32
    bdt = mybir.dt.bfloat16
    KMAX = max(KT)

    nc = bacc.Bacc("TRN2", num_devices=N_CORES)
    xt1_in = nc.dram_tensor("xt1", [131, SHARD], bdt, kind="ExternalInput")
    w_ins = []
    for li, (cin, H, D) in enumerate(_LAYERS):
        kf = 2 if cin + 1 > 128 else 1
        w_ins.append(
            nc.dram_tensor(f"w{li+1}", [128, kf, 4 * H * D], bdt, kind="ExternalInput")
        )
    idx_ins, mask_ins = [], []
    for b, (t0, t1, Kb) in enumerate(BANDS):
        rows = (t1 - t0) * P
        idx_ins.append(
            nc.dram_tensor(f"idx{b}", [rows, Kb], mybir.dt.int32, kind="ExternalInput")
        )
        mask_ins.append(
            nc.dram_tensor(f"mask{b}", [rows, Kb], bdt, kind="ExternalInput")
        )
    band_of = []
    for t in range(NT):
        for b, (t0, t1, Kb) in enumerate(BANDS):
            if t0 <= t < t1:
                band_of.append(b)
                break
    y_out = nc.dram_tensor("y", [SHARD, 40], bdt, kind="ExternalOutput")

    with tile.TileContext(nc) as tc:
        with (
            tc.tile_pool(name="wpool", bufs=1) as wpool,
            tc.tile_pool(name="sb", bufs=2) as sb,
            tc.tile_pool(name="gp", bufs=2) as gp,
            tc.tile_pool(name="res", bufs=1) as res,
            tc.tile_pool(name="psum", bufs=4, space="PSUM") as pspool,
            tc.tile_pool(name="dram", bufs=1, space="DRAM") as dram,
        ):
            wts = []
            for li, (cin, H, D) in enumerate(_LAYERS):
                kf = 2 if cin + 1 > 128 else 1
                wt = wpool.tile([128, kf, 4 * H * D], bdt, tag=f"w{li}")
                nc.sync.dma_start(out=wt[:], in_=w_ins[li][:])
                wts.append(wt)

            h_prev = None  # DRAM [SHARD_PAD, F_prev + 1] bf16 (ones col last)
            for li, (cin, H, D) in enumerate(_LAYERS):
                F = H * D
                FQ = F // 2  # f32 words holding the bf16 q row
                ELEM = 2 * F
                cr = cin + 1
                kf = 2 if cr > 128 else 1
                m4 = 4 * F
                wt = wts[li]

                kv_local = dram.tile([SHARD, ELEM], bdt, tag=f"kvl{li}")
                kv_full = dram.tile([N_NODES, ELEM], bdt, tag=f"kvf{li}")
                qres = res.tile([P, NT * F], bdt, tag="qres")
                sres = res.tile([P, NT * F], fdt, tag="sres")
                h_tab = None
                if li < 2:
                    h_tab = dram.tile([SHARD_PAD, F + 1], bdt, tag=f"ht{li}")

                # -------- pass A: projections for own shard --------
                NCH = []
                c0 = 0
                while c0 < m4:
                    cn = min(400, m4 - c0)
                    NCH.append((c0, cn))
                    c0 += cn
                for t in range(NT):
                    m0 = t * P
                    m = min(P, SHARD - m0)
                    xt_t = sb.tile([128, kf, P], bdt, tag="xt")
                    nc.vector.memset(xt_t[:], 0)
                    if li == 0:
                        nc.sync.dma_start(
                            out=xt_t[:128, 0, :m], in_=xt1_in[0:128, m0 : m0 + m]
                        )
                        nc.sync.dma_start(
                            out=xt_t[0:3, 1, :m], in_=xt1_in[128:131, m0 : m0 + m]
                        )
                    else:
                        # h_prev has a ones column at index fp: the transpose
                        # loads features AND the bias ones-row together.
                        fp = _LAYERS[li - 1][1] * _LAYERS[li - 1][2]
                        r1 = min(fp + 1, 128)
                        nc.sync.dma_start_transpose(
                            out=xt_t[:r1, 0, :], in_=h_prev[m0 : m0 + P, 0:r1]
                        )
                        if fp + 1 > 128:
                            nc.sync.dma_start_transpose(
                                out=xt_t[: fp + 1 - 128, 1, :],
                                in_=h_prev[m0 : m0 + P, 128 : fp + 1],
                            )
                    kvb = sb.tile([P, ELEM], bdt, tag="kvb")
                    # W columns are ordered q|s|k|v so k|v is one contiguous copy
                    for (c0, cn) in NCH:
                        ps = pspool.tile([P, 400], fdt, tag="ps")
                        for ki in range(kf):
                            nc.tensor.matmul(
                                ps[:m, :cn],
                                lhsT=xt_t[:, ki, :m],
                                rhs=wt[:, ki, c0 : c0 + cn],
                                start=(ki == 0),
                                stop=(ki == kf - 1),
                            )
                        for dst_ap, soff, w_ in (
                            (qres[:m, t * F : (t + 1) * F], 0, F),
                            (sres[:m, t * F : (t + 1) * F], F, F),
                            (kvb[:m, :], 2 * F, 2 * F),
                        ):
                            lo = max(soff, c0)
                            hi = min(soff + w_, c0 + cn)
                            if lo < hi:
                                nc.vector.tensor_copy(
                                    out=dst_ap[:, lo - soff : hi - soff],
                                    in_=ps[:m, lo - c0 : hi - c0],
                                )
                    nc.sync.dma_start(out=kv_local[m0 : m0 + m, :], in_=kvb[:m, :])

                # -------- AllGather the k|v table --------
                nc.gpsimd.collective_compute(
                    "AllGather",
                    mybir.AluOpType.bypass,
                    replica_groups=[list(range(N_CORES))],
                    ins=[kv_local[:]],
                    outs=[kv_full[:]],
                )

                # -------- pass B: attention, batched over G tiles --------
                for (bt0, G, b) in BATCHES:
                    Kb = BANDS[b][2]
                    GK = G * Kb
                    m0 = bt0 * P
                    mlast = min(P, SHARD - (bt0 + G - 1) * P)
                    full = GK if mlast == P else (G - 1) * Kb  # cols with all 128 rows valid
                    r0b = (bt0 - BANDS[b][0]) * P
                    it = sb.tile([P, GK], mybir.dt.int32, tag="it")
                    nc.sync.dma_start(
                        out=it[:].rearrange("p (g k) -> p g k", g=G),
                        in_=idx_ins[b][r0b : r0b + G * P, :].rearrange(
                            "(g p) k -> p g k", g=G
                        ),
                    )
                    mt = sb.tile([P, GK], bdt, tag="mt")
                    nc.sync.dma_start(
                        out=mt[:].rearrange("p (g k) -> p g k", g=G),
                        in_=mask_ins[b][r0b : r0b + G * P, :].rearrange(
                            "(g p) k -> p g k", g=G
                        ),
                    )
                    qt = qres[:, bt0 * F : (bt0 + G) * F]
                    st = sres[:, bt0 * F : (bt0 + G) * F]

                    g2 = gp.tile([P, GK, ELEM], bdt, tag="g")
                    for c in range(GK):
                        nc.gpsimd.indirect_dma_start(
                            out=g2[:, c, :],
                            out_offset=None,
                            in_=kv_full[:],
                            in_offset=bass.IndirectOffsetOnAxis(
                                ap=it[:, c : c + 1], axis=0
                            ),
                        )
                    # scores: in-place q*k product over the k half, then reduce
                    nc.vector.tensor_tensor(
                        out=g2[:, :, 0:F].rearrange("p (g k) f -> p g k f", g=G),
                        in0=g2[:, :, 0:F].rearrange("p (g k) f -> p g k f", g=G),
                        in1=qt.rearrange("p (g f) -> p g f", g=G)
                        .unsqueeze(2)
                        .broadcast_to([P, G, Kb, F]),
                        op=mybir.AluOpType.mult,
                    )
                    scores = sb.tile([P, GK * H], fdt, tag="scores")
                    nc.vector.tensor_reduce(
                        out=scores[:],
                        in_=g2[:, :, 0:F].rearrange("p c (h d) -> p c h d", h=H),
                        axis=mybir.AxisListType.X,
                        op=mybir.AluOpType.add,
                    )
                    sm = sb.tile([P, GK * H], fdt, tag="sm")
                    nc.vector.scalar_tensor_tensor(
                        out=sm[:].rearrange("p (c h) -> p c h", h=H),
                        in0=scores[:].rearrange("p (c h) -> p c h", h=H),
                        scalar=60.0,
                        in1=mt[:].to_broadcast([P, GK, H]),
                        op0=mybir.AluOpType.min,
                        op1=mybir.AluOpType.add,
                    )
                    es = sb.tile([P, GK * H], bdt, tag="es")
                    nc.scalar.activation(
                        out=es[:], in_=sm[:], func=mybir.ActivationFunctionType.Exp
                    )
                    dn = sb.tile([P, G * H], fdt, tag="dn")
                    nc.vector.tensor_reduce(
                        out=dn[:],
                        in_=es[:].rearrange("p (g k h) -> p g h k", g=G, k=Kb),
                        axis=mybir.AxisListType.X,
                        op=mybir.AluOpType.add,
                    )
                    # weighted v in place over the v half
                    nc.vector.tensor_tensor(
                        out=g2[:, :, F:ELEM].rearrange("p c (h d) -> p c h d", h=H),
                        in0=g2[:, :, F:ELEM].rearrange("p c (h d) -> p c h d", h=H),
                        in1=es[:]
                        .rearrange("p (c h) -> p c h", h=H)
                        .unsqueeze(3)
                        .broadcast_to([P, GK, H, D]),
                        op=mybir.AluOpType.mult,
                    )
                    osum = sb.tile([P, G * F], fdt, tag="osum")
                    nc.vector.tensor_reduce(
                        out=osum[:],
                        in_=g2[:, :, F:ELEM].rearrange("p (g k) f -> p g f k", g=G),
                        axis=mybir.AxisListType.X,
                        op=mybir.AluOpType.add,
                    )
                    rec = sb.tile([P, G * H], fdt, tag="rec")
                    nc.vector.reciprocal(out=rec[:], in_=dn[:])
                    hsb = sb.tile([P, G * F], fdt, tag="hsb")
                    nc.vector.tensor_tensor(
                        out=hsb[:].rearrange("p (g h d) -> p g h d", g=G, h=H),
                        in0=osum[:].rearrange("p (g h d) -> p g h d", g=G, h=H),
                        in1=rec[:]
                        .rearrange("p (g h) -> p g h", g=G)
                        .unsqueeze(3)
                        .broadcast_to([P, G, H, D]),
                        op=mybir.AluOpType.mult,
                    )
                    nc.vector.tensor_tensor(
                        out=hsb[:], in0=hsb[:], in1=st, op=mybir.AluOpType.add
                    )
                    if li < 2:
                        hb = sb.tile([P, G * (F + 1)], bdt, tag="hb")
                        nc.vector.scalar_tensor_tensor(
                            out=hb[:].rearrange("p (g f) -> p g f", g=G)[:, :, 0:F],
                            in0=hsb[:].rearrange("p (g f) -> p g f", g=G),
                            scalar=LEAKY_ALPHA,
                            in1=hsb[:].rearrange("p (g f) -> p g f", g=G),
                            op0=mybir.AluOpType.mult,
                            op1=mybir.AluOpType.max,
                        )
                        nc.vector.memset(
                            hb[:].rearrange("p (g f) -> p g f", g=G)[:, :, F : F + 1],
                            1.0,
                        )
                        if mlast == P:
                            nc.sync.dma_start(
                                out=h_tab[m0 : m0 + G * P, :].rearrange(
                                    "(g p) f -> p g f", g=G
                                ),
                                in_=hb[:].rearrange("p (g f) -> p g f", g=G),
                            )
                        else:
                            # ragged tile is always its own G=1 batch
                            nc.sync.dma_start(
                                out=h_tab[m0 : m0 + mlast, :], in_=hb[:mlast, :]
                            )
                    else:
                        negm = sb.tile([P, G], fdt, tag="negm")
                        nc.vector.tensor_reduce(
                            out=negm[:],
                            in_=hsb[:].rearrange("p (g f) -> p g f", g=G),
                            axis=mybir.AxisListType.X,
                            op=mybir.AluOpType.max,
                            negate=True,
                        )
                        z = sb.tile([P, G * F], fdt, tag="z")
                        nc.vector.tensor_tensor(
                            out=z[:].rearrange("p (g f) -> p g f", g=G),
                            in0=hsb[:].rearrange("p (g f) -> p g f", g=G),
                            in1=negm[:].unsqueeze(2).broadcast_to([P, G, F]),
                            op=mybir.AluOpType.add,
                        )
                        ez = sb.tile([P, G * F], fdt, tag="ez")
                        nc.scalar.activation(
                            out=ez[:], in_=z[:], func=mybir.ActivationFunctionType.Exp
                        )
                        se = sb.tile([P, G], fdt, tag="se")
                        nc.vector.tensor_reduce(
                            out=se[:],
                            in_=ez[:].rearrange("p (g f) -> p g f", g=G),
                            axis=mybir.AxisListType.X,
                            op=mybir.AluOpType.add,
                        )
                        ls = sb.tile([P, G], fdt, tag="ls")
                        nc.scalar.activation(
                            out=ls[:], in_=se[:], func=mybir.ActivationFunctionType.Ln
                        )
                        out_t = sb.tile([P, G * F], bdt, tag="out")
                        nc.vector.tensor_tensor(
                            out=out_t[:].rearrange("p (g f) -> p g f", g=G),
                            in0=z[:].rearrange("p (g f) -> p g f", g=G),
                            in1=ls[:].unsqueeze(2).broadcast_to([P, G, F]),
                            op=mybir.AluOpType.subtract,
                        )
                        if mlast == P:
                            nc.sync.dma_start(
                                out=y_out[m0 : m0 + G * P, :].rearrange(
                                    "(g p) f -> p g f", g=G
                                ),
                                in_=out_t[:].rearrange("p (g f) -> p g f", g=G),
                            )
                        else:
                            nc.sync.dma_start(
                                out=y_out[m0 : m0 + mlast, :], in_=out_t[:mlast, :]
                            )
                h_prev = h_tab
    nc.compile()
    return nc


def _prep_structure(src, dst):
    """Degree-sorted per-core slot tables.

    Returns (banded idx/mask per core, KT, bands, perm) where
    perm maps new (degree-sorted) global node id -> old global node id.
    """
    deg = np.bincount(dst, minlength=N_NODES)
    assert deg.min() >= 1, "zero in-degree node: reciprocal needs the epsilon path"
    perm = np.empty(N_NODES, np.int64)
    for c in range(N_CORES):
        sl = slice(c * SHARD, (c + 1) * SHARD)
        order_c = np.argsort(-deg[sl], kind="stable")
        perm[sl] = c * SHARD + order_c
    inv_perm = np.empty(N_NODES, np.int64)
    inv_perm[perm] = np.arange(N_NODES)

    ndst = inv_perm[dst]
    nsrc = inv_perm[src]
    order = np.argsort(ndst, kind="stable")
    dsorted = ndst[order]
    ssorted = nsrc[order]
    ndeg = np.bincount(dsorted, minlength=N_NODES)
    starts = np.zeros(N_NODES + 1, np.int64)
    np.cumsum(ndeg, out=starts[1:])
    rank = np.arange(dsorted.shape[0], dtype=np.int64) - starts[dsorted]

    dmat = ndeg.reshape(N_CORES, SHARD)
    KT = []
    for t in range(NT):
        hi = min((t + 1) * P, SHARD)
        KT.append(max(1, int(dmat[:, t * P : hi].max())))

    KMAX = max(KT)
    idx = np.zeros((N_NODES, KMAX), np.int32)
    maskb = np.full((N_NODES, KMAX), -30000.0, np.float32)
    idx[dsorted, rank] = ssorted.astype(np.int32)
    maskb[dsorted, rank] = 0.0

    # width bands over the (non-increasing) KT: 4 bands minimizing padded area
    nb = 4
    INF = 1 << 60
    cost = [[INF] * (nb + 1) for _ in range(NT + 1)]
    prevb = [[-1] * (nb + 1) for _ in range(NT + 1)]
    cost[0][0] = 0
    for t1 in range(1, NT + 1):
        for b in range(1, nb + 1):
            for t0 in range(t1):
                if cost[t0][b - 1] == INF:
                    continue
                w = KT[t0] * (t1 - t0)  # KT non-increasing: band width = KT[t0]
                if cost[t0][b - 1] + w < cost[t1][b]:
                    cost[t1][b] = cost[t0][b - 1] + w
                    prevb[t1][b] = t0
    bands = []
    t1, b = NT, nb
    while t1 > 0:
        t0 = prevb[t1][b]
        bands.append((t0, t1, KT[t0]))
        t1, b = t0, b - 1
    bands.reverse()

    import ml_dtypes

    maskh = maskb.astype(ml_dtypes.bfloat16)
    idx_bc = []   # idx_bc[c][b], mask_bc[c][b]
    mask_bc = []
    for c in range(N_CORES):
        ib_list, mb_list = [], []
        for (t0, t1, Kb) in bands:
            rows = (t1 - t0) * P
            lo = c * SHARD + t0 * P
            hi = min(c * SHARD + t1 * P, (c + 1) * SHARD)
            blk_i = np.zeros((rows, Kb), np.int32)
            blk_m = np.full((rows, Kb), -30000.0, np.float32).astype(ml_dtypes.bfloat16)
            blk_i[: hi - lo] = idx[lo:hi, :Kb]
            blk_m[: hi - lo] = maskh[lo:hi, :Kb]
            ib_list.append(np.ascontiguousarray(blk_i))
            mb_list.append(np.ascontiguousarray(blk_m))
        idx_bc.append(ib_list)
        mask_bc.append(mb_list)
    # batches of consecutive tiles within a band: G*Kb bounded by SBUF budget
    ELEM1 = 2 * _LAYERS[0][1] * _LAYERS[0][2]
    batches = []
    for b, (t0, t1, Kb) in enumerate(bands):
        t = t0
        while t < t1:
            G = 1
            while (
                t + G < t1
                and (G + 1) * Kb * ELEM1 * 2 <= 45056  # 44KB/partition for g2
                and G < 8
                and t + G != NT - 1  # keep the ragged last tile in its own batch
            ):
                G += 1
            if t == NT - 1 or t + G > NT - 1:
                G = min(G, max(1, NT - 1 - t)) if t < NT - 1 else 1
            batches.append((t, G, b))
            t += G
    return idx_bc, mask_bc, KT, bands, batches, perm


def _fold_w(W4, b4, cin, scale_q, F):
    # W4/b4 arrive in q|s|k|v column order; scale applies to the q block
    import ml_dtypes

    kf = 2 if cin + 1 > 128 else 1
    w = np.zeros((128 * kf, 4 * F), np.float32)
    w[:cin] = W4
    w[cin] = b4
    w[:, 0:F] *= scale_q
    return (
        w.astype(ml_dtypes.bfloat16).reshape(kf, 128, 4 * F).transpose(1, 0, 2).copy()
    )


def _get_program(KT, BANDS, BATCHES):
    key = (tuple(KT), tuple(BANDS), tuple(BATCHES))
    if _STATE.get("key") != key:
        nc = _build_program(KT, BANDS, BATCHES)
        import ml_dtypes
        from concourse import bass2jax

        dummy = []
        for _ in range(N_CORES):
            d = {
                "xt1": np.zeros((131, SHARD), ml_dtypes.bfloat16),
            }
            for b, (t0, t1, Kb) in enumerate(BANDS):
                rows = (t1 - t0) * P
                d[f"idx{b}"] = np.zeros((rows, Kb), np.int32)
                d[f"mask{b}"] = np.full(
                    (rows, Kb), -30000.0, np.float32
                ).astype(ml_dtypes.bfloat16)
            for li, (cin, H, D) in enumerate(_LAYERS):
                kf = 2 if cin + 1 > 128 else 1
                d[f"w{li+1}"] = np.zeros((128, kf, 4 * H * D), ml_dtypes.bfloat16)
            dummy.append(d)
        bass2jax.run_bass_via_pjrt(nc, dummy, n_cores=N_CORES)
        _STATE["key"] = key
        _STATE["nc"] = nc
    return _STATE["nc"]


def kernel(**inputs):
    import ml_dtypes
    from concourse.bass_utils import run_bass_kernel_spmd

    x = np.asarray(inputs["x"], np.float32)
    edge_index = np.asarray(inputs["edge_index"])
    src = edge_index[0].astype(np.int64)
    dst = edge_index[1].astype(np.int64)

    idx_bc, mask_bc, KT, bands, batches, perm = _prep_structure(src, dst)
    nc = _get_program(KT, bands, batches)

    ws = []
    for li, (cin, H, D) in enumerate(_LAYERS):
        W4 = np.concatenate(
            [
                np.asarray(inputs[f"W{nm}{li+1}"], np.float32)
                for nm in ["q", "s", "k", "v"]
            ],
            axis=1,
        )
        b4 = np.concatenate(
            [
                np.asarray(inputs[f"b{nm}{li+1}"], np.float32)
                for nm in ["q", "s", "k", "v"]
            ]
        )
        ws.append(_fold_w(W4, b4, cin, 1.0 / np.sqrt(np.float32(D)), H * D))

    xp = x[perm]  # rows in new (degree-sorted) order
    xb = xp.astype(ml_dtypes.bfloat16)
    ones = np.ones((1, SHARD), ml_dtypes.bfloat16)
    in_maps = []
    for c in range(N_CORES):
        sl = slice(c * SHARD, (c + 1) * SHARD)
        xt1 = np.ascontiguousarray(np.concatenate([xb[sl].T, ones], axis=0))
        m = {"xt1": xt1}
        for b in range(len(bands)):
            m[f"idx{b}"] = idx_bc[c][b]
            m[f"mask{b}"] = mask_bc[c][b]
        for li in range(3):
            m[f"w{li+1}"] = ws[li]
        in_maps.append(m)

    import time as _time

    t0 = _time.time()
    res = run_bass_kernel_spmd(nc, in_maps, list(range(N_CORES)))
    dt = int((_time.time() - t0) * 1e9)
    globals()["_DEVICE_WALL_NS"] = globals().get("_DEVICE_WALL_NS", 0) + dt
    globals().setdefault("_LAUNCH_NS", []).append(dt)

    y_perm = np.concatenate(
        [res.results[c]["y"].astype(np.float32) for c in range(N_CORES)], axis=0
    )
    y = np.empty_like(y_perm)
    y[perm] = y_perm  # un-permute rows back to original node order
    return y
